# revision 6
# baseline (speedup 1.0000x reference)
"""Multi-head attention (B=4, S=2048, D=512, H=8) on 8 trn2 NeuronCores.

Sharding: core c = (batch b = c//2, query-half qh = c%2). Each core computes
the full attention output for 1024 query rows of one batch element.

v2: single fully-pipelined phase. The attention inner loop is ACT-bound
(exp of [128,1024] logit tiles, ~1.1us each, 72 tiles); all projection and
O-projection matmul work is injected into the PE stream between the logits
pair of iteration t and the AV pair of iteration t-1, filling the PE's
exp-wait slack. Head-pair 0's projections run up front so the first exp
fires as soon as ~2MB of input has landed; DMA pieces are issued in
consumption order. Warm-up matmuls keep the PE HAM un-throttled from ~6us.

Device-side scheme (unchanged from v1):
  - host supplies transposed activations (X^T = [D, S] layouts)
  - Q^T, K^T projections: lhsT = weight chunk, rhs = X^T chunk; per-partition
    bias added during the PSUM->SBUF drain.
  - V projection in natural [s, dout] layout, stored [128, H, DH+1] with a
    ones column per head (softmax denominator rides in the AV matmul, M=65).
  - logits transposed: lg^T[s_k, q] = K_h^T-block . Q_h^T, two heads per PE
    pass (row-packed K=64 at partitions 0-63 / 64-127, truly concurrent).
  - exp on ACT: w = exp(0.125*lg + mb); masked/padded keys get bias -1e9.
  - AV: attnU^T[d, q] accumulated per head (M=65 incl. denominator row).
  - normalization: reciprocal (DVE) + partition broadcast (gpsimd) + mul.
  - O projection: out[q, dout], lhsT = attnN^T; streamed to HBM per 128 rows.

Masked keys (mask==1) are compacted away on the host (exact: their softmax
weight is 0), keys padded to a multiple of 128 with -1e9 mask bias.
"""

import os
import numpy as np

B, S, D, H = 4, 2048, 512, 8
DH = D // H
NCORE = 8
SQ = S // 2  # queries per core
SCALE = 1.0 / float(np.sqrt(DH))

_BUILT = {}


def _chunks(total, step):
    out = []
    c0 = 0
    while c0 < total:
        out.append((c0, min(step, total - c0)))
        c0 += step
    return out


def build_bass(s_pad, mm_dtype="bf16"):
    import concourse.bass as bass  # noqa: F401
    import concourse.mybir as mybir
    import concourse.tile as tile
    from concourse import bacc
    from contextlib import ExitStack

    f32 = mybir.dt.float32
    mmdt = {
        "bf16": mybir.dt.bfloat16,
        "f32r": mybir.dt.float32r,
        "f32": mybir.dt.float32,
    }[mm_dtype]
    EXP = mybir.ActivationFunctionType.Exp

    nsk = s_pad // 128

    nc = bacc.Bacc(
        "TRN2",
        target_bir_lowering=False,
        debug=False,
        enable_asserts=False,
        num_devices=NCORE,
    )

    KW, QW = 4 * s_pad, 4 * SQ
    d_bk_blob = nc.dram_tensor("blob_k", [128, 2048 + KW], mmdt, kind="ExternalInput").ap()
    d_bq_blob = nc.dram_tensor("blob_q", [128, 2048 + QW], mmdt, kind="ExternalInput").ap()
    d_bv_blob = nc.dram_tensor("blob_v", [128, 2048 + KW + D + 8], mmdt, kind="ExternalInput").ap()
    d_bo_blob = nc.dram_tensor("blob_o", [128, 2048 + D], mmdt, kind="ExternalInput").ap()
    d_mb = nc.dram_tensor("mb", [128, nsk], f32, kind="ExternalInput").ap()
    d_bq = nc.dram_tensor("bq_pp", [128, 4], f32, kind="ExternalInput").ap()
    d_bk = nc.dram_tensor("bk_pp", [128, 4], f32, kind="ExternalInput").ap()
    d_out = nc.dram_tensor("out", [SQ, D], f32, kind="ExternalOutput").ap()

    with tile.TileContext(nc) as tc, ExitStack() as ctx, nc.allow_low_precision(
        "matmul operands held in bf16 (tolerance 2e-2; measured ~6e-3)"
    ):
        sb = ctx.enter_context(tc.tile_pool(name="sb", bufs=1))
        # PSUM budget (8 banks): lg 2x[128,1024]f32 = 4, pj 2x[128,512]f32 = 2,
        # avA/avB 1 buf each [65,512]f32 = 2.
        ps_lg = ctx.enter_context(tc.tile_pool(name="pslg", bufs=2, space="PSUM"))
        ps_pj = ctx.enter_context(tc.tile_pool(name="pspj", bufs=2, space="PSUM"))
        ps_av = ctx.enter_context(tc.tile_pool(name="psav", bufs=1, space="PSUM"))
        wexp_p = ctx.enter_context(tc.tile_pool(name="wexp", bufs=4))
        osb_p = ctx.enter_context(tc.tile_pool(name="osb", bufs=2))
        r_p = ctx.enter_context(tc.tile_pool(name="rp", bufs=2))

        # ---- persistent SBUF tiles ----
        blk = sb.tile([128, 2048 + KW], mmdt, tag="blk", name="blk")
        blq = sb.tile([128, 2048 + QW], mmdt, tag="blq", name="blq")
        blv = sb.tile([128, 2048 + KW + D + 8], mmdt, tag="blv", name="blv")
        blo = sb.tile([128, 2048 + D], mmdt, tag="blo", name="blo")
        bk = sb.tile([128, 4], f32, tag="bk", name="bk")
        bq = sb.tile([128, 4], f32, tag="bq", name="bq")
        mb = sb.tile([128, nsk], f32, tag="mb", name="mb")
        kT = [sb.tile([128, s_pad], mmdt, tag=f"kT{j}", name=f"kT{j}") for j in range(4)]
        qT = [sb.tile([128, SQ], mmdt, tag=f"qT{j}", name=f"qT{j}") for j in range(4)]
        attnN = [
            sb.tile([128, SQ], mmdt, tag=f"attnN{pr}", name=f"attnN{pr}")
            for pr in range(4)
        ]
        v = [
            sb.tile([128, H, DH + 1], mmdt, tag=f"v{t}", name=f"v{t}")
            for t in range(nsk)
        ]
        wu = sb.tile([128, 512], mmdt, tag="wu", name="wu")

        bvb = blv[:, 2048 + KW : 2048 + KW + D]
        ones8 = blv[:, 2048 + KW + D : 2048 + KW + D + 8]
        bob = blo[:, 2048 : 2048 + D]

        # ---- DMA emission, consumption order ----
        X = 2048  # x-region offset inside k/q/v blobs
        h_k = min(((s_pad // 2 + 127) // 128) * 128, s_pad)  # 576 for 1152
        p_v0 = min(5 * 128, s_pad)  # xv piece 0 covers t=0..4

        def ld(dst, src):
            nc.sync.dma_start(dst, src)

        # weights wk, wq (2 pieces each)
        for c0, cw in _chunks(2048, 1024):
            ld(blk[:, c0 : c0 + cw], d_bk_blob[:, c0 : c0 + cw])
        for c0, cw in _chunks(2048, 1024):
            ld(blq[:, c0 : c0 + cw], d_bq_blob[:, c0 : c0 + cw])
        ld(bk[:], d_bk[:])
        ld(bq[:], d_bq[:])
        ld(mb[:], d_mb[:])
        # xk first halves (per dk)
        for dk in range(4):
            o = X + dk * s_pad
            ld(blk[:, o : o + h_k], d_bk_blob[:, o : o + h_k])
        # xv piece 0 (t=0..4) + wv — needed by av(t) from the first window
        for dk in range(4):
            o = X + dk * s_pad
            ld(blv[:, o : o + p_v0], d_bv_blob[:, o : o + p_v0])
        for c0, cw in _chunks(2048, 1024):
            ld(blv[:, c0 : c0 + cw], d_bv_blob[:, c0 : c0 + cw])
        # v bias + ones columns
        ld(blv[:, X + KW :], d_bv_blob[:, X + KW :])
        # xq first halves
        for dk in range(4):
            o = X + dk * SQ
            ld(blq[:, o : o + 512], d_bq_blob[:, o : o + 512])
        # xk second halves
        if h_k < s_pad:
            for dk in range(4):
                o = X + dk * s_pad + h_k
                e = X + dk * s_pad + s_pad
                ld(blk[:, o:e], d_bk_blob[:, o:e])
        # xv piece 1
        if p_v0 < s_pad:
            for dk in range(4):
                o = X + dk * s_pad + p_v0
                e = X + dk * s_pad + s_pad
                ld(blv[:, o:e], d_bv_blob[:, o:e])
        # wo + output bias
        for c0, cw in _chunks(2048 + D, 1024):
            ld(blo[:, c0 : c0 + cw], d_bo_blob[:, c0 : c0 + cw])
        # xq second halves (needed from window (1,0), ~55us in)
        for dk in range(4):
            o = X + dk * SQ + 512
            ld(blq[:, o : o + 512], d_bq_blob[:, o : o + 512])

        # ---- PE warm-up (no data deps; runs during preamble/DMA) ----
        nc.vector.memset(wu[:], 0.0)
        for i in range(14):
            wps = ps_pj.tile([128, 512], f32, tag="pj", name=f"wups{i}")
            nc.tensor.matmul(wps[:], lhsT=wu[:, 0:128], rhs=wu[:], start=True, stop=True)

        # ---- projection emitters ----
        def kq_chunk(which, j, c0, cw):
            blob, bias_t, dst = (blk, bk, kT) if which == "k" else (blq, bq, qT)
            sp = s_pad if which == "k" else SQ
            ps = ps_pj.tile([128, 512], f32, tag="pj", name=f"pj{which}{j}_{c0}")
            for dk in range(4):
                nc.tensor.matmul(
                    ps[:, 0:cw],
                    lhsT=blob[:, dk * 512 + j * 128 : dk * 512 + (j + 1) * 128],
                    rhs=blob[:, X + dk * sp + c0 : X + dk * sp + c0 + cw],
                    start=(dk == 0),
                    stop=(dk == 3),
                )
            nc.vector.tensor_scalar_add(
                dst[j][:, c0 : c0 + cw], ps[:, 0:cw], bias_t[:, j : j + 1]
            )

        def v_proj(t):
            ps = ps_pj.tile([128, 512], f32, tag="pj", name=f"pjv{t}")
            for dk in range(4):
                nc.tensor.matmul(
                    ps[:],
                    lhsT=blv[:, X + dk * s_pad + t * 128 : X + dk * s_pad + (t + 1) * 128],
                    rhs=blv[:, dk * 512 : (dk + 1) * 512],
                    start=(dk == 0),
                    stop=(dk == 3),
                )
            nc.vector.tensor_copy(
                v[t][:, :, DH : DH + 1], ones8.rearrange("p (h o) -> p h o", o=1)
            )
            nc.vector.scalar_tensor_tensor(
                v[t][:, :, 0:DH],
                ps[:].rearrange("p (h d) -> p h d", h=H),
                1.0,
                bvb.rearrange("p (h d) -> p h d", h=H),
                op0=mybir.AluOpType.mult,
                op1=mybir.AluOpType.add,
            )

        def o_unit(qc, qt):
            qq = qc * 512 + qt * 128
            ops = ps_pj.tile([128, D], f32, tag="pj", name=f"ops{qc}_{qt}")
            for pr2 in range(4):
                nc.tensor.matmul(
                    ops[:],
                    lhsT=attnN[pr2][:, qq : qq + 128],
                    rhs=blo[:, pr2 * 512 : (pr2 + 1) * 512],
                    start=(pr2 == 0),
                    stop=(pr2 == 3),
                )
            osb = osb_p.tile([128, D], f32, tag="osb", name=f"osb{qc}_{qt}")
            nc.vector.scalar_tensor_tensor(
                osb[:],
                ops[:],
                1.0,
                bob,
                op0=mybir.AluOpType.mult,
                op1=mybir.AluOpType.add,
            )
            nc.sync.dma_start(d_out[qq : qq + 128, :], osb[:])

        # ---- up-front projections: first K/Q chunks + v[0..4] ----
        # (everything else is injected into the attention windows)
        kch = _chunks(s_pad, 512)
        kq_chunk("k", 0, 0, 512)
        for t in range(min(5, nsk)):
            v_proj(t)
        kq_chunk("q", 0, 0, 512)

        # ---- injection schedule ----
        # Ordered queue of remaining PE work units, consumed into per-t slots
        # of the attention windows. Slot capacities: 0 for the first 3 slots
        # of window (0,0) (input DMA still in flight), 2 for its last 4 slots
        # (so head-pair 1's first chunks land before window (0,1) starts).
        main_q = []
        for c0, cw in kch[1:]:
            main_q.append(("k", 0, c0, cw))
        for t in range(5, nsk):
            main_q.append(("v", t))
        for j in (1, 2, 3):
            main_q.append(("q", j, 0, 512))
            for c0, cw in kch:
                main_q.append(("k", j, c0, cw))
        main_q.append(("q", 0, 512, 512))
        for j in (1, 2, 3):
            main_q.append(("q", j, 512, 512))

        def run_unit(u):
            if u[0] == "v":
                v_proj(u[1])
            elif u[0] in ("k", "q"):
                kq_chunk(u[0], u[1], u[2], u[3])
            elif u[0] == "o":
                o_unit(u[1], u[2])

        windows = [(qc, pr) for qc in range(2) for pr in range(4)]
        slot_plan = {w: [[] for _ in range(nsk)] for w in windows}
        cap = {w: [1] * nsk for w in windows}
        for sl in range(min(3, nsk)):
            cap[(0, 0)][sl] = 0
        for sl in range(max(3, nsk - 2), nsk):
            cap[(0, 0)][sl] = 2
        # pin O(0) into window (1,0) slots 3..6 (must follow finish(0,3))
        for qt in range(4):
            sl = min(3 + qt, nsk - 1)
            slot_plan[(1, 0)][sl].append(("o", 0, qt))
            cap[(1, 0)][sl] = 0

        def unit_ns(u):
            if u[0] in ("k", "q"):
                return 860 if u[3] >= 512 else 290
            return 900

        # fill greedily, capping injected PE work per window (~ACT slack)
        budget = {w: (99000 if w == (0, 0) else 4000) for w in windows}
        fill = []
        for w in windows:
            for sl in range(nsk):
                for _ in range(cap[w][sl]):
                    fill.append((w, sl))
        pos = 0
        for u in main_q:
            while pos < len(fill):
                w, sl = fill[pos]
                if budget[w] >= unit_ns(u):
                    break
                while pos < len(fill) and fill[pos][0] == w:
                    pos += 1
            if pos < len(fill):
                w, sl = fill[pos]
                slot_plan[w][sl].append(u)
                budget[w] -= unit_ns(u)
                pos += 1
            else:
                slot_plan[windows[-1]].append([u])  # overflow: after last window

        # ---- attention windows ----
        def window(qc, pr):
            q0 = qc * 512
            hA, hB = 2 * pr, 2 * pr + 1
            avA = ps_av.tile([65, 512], f32, tag="avA", name=f"avA{qc}{pr}")
            avB = ps_av.tile([65, 512], f32, tag="avB", name=f"avB{qc}{pr}")
            plan = slot_plan[(qc, pr)]
            pend = None

            def emit_av(t, wx):
                last = t == nsk - 1
                nc.tensor.matmul(
                    avA[0:65, :],
                    lhsT=v[t][:, hA : hA + 1, 0 : DH + 1],
                    rhs=wx[:, 0:512],
                    start=(t == 0),
                    stop=last,
                )
                nc.tensor.matmul(
                    avB[0:65, :],
                    lhsT=v[t][:, hB : hB + 1, 0 : DH + 1],
                    rhs=wx[:, 512:1024],
                    start=(t == 0),
                    stop=last,
                )

            for t in range(nsk):
                lg = ps_lg.tile([128, 1024], f32, tag="lg", name=f"lg{qc}{pr}_{t}")
                nc.tensor.matmul(
                    lg[:, 0:512],
                    lhsT=kT[pr][0:64, t * 128 : (t + 1) * 128],
                    rhs=qT[pr][0:64, q0 : q0 + 512],
                    start=True,
                    stop=True,
                )
                nc.tensor.matmul(
                    lg[:, 512:1024],
                    lhsT=kT[pr][64:128, t * 128 : (t + 1) * 128],
                    rhs=qT[pr][64:128, q0 : q0 + 512],
                    start=True,
                    stop=True,
                )
                wx = wexp_p.tile([128, 1024], mmdt, tag="wexp", name=f"wx{qc}{pr}_{t}")
                nc.scalar.activation(wx[:], lg[:], EXP, bias=mb[:, t : t + 1], scale=SCALE)
                if pend is not None:
                    emit_av(*pend)
                for u in plan[t]:
                    run_unit(u)
                pend = (t, wx)
            emit_av(*pend)
            for lst in plan[nsk:]:  # overflow units (last window only)
                for u in lst:
                    run_unit(u)

            # normalization chain
            dsA = r_p.tile([1, 512], f32, tag="dsA", name=f"dsA{qc}{pr}")
            dsB = r_p.tile([1, 512], f32, tag="dsB", name=f"dsB{qc}{pr}")
            nc.vector.tensor_copy(dsA[0:1, :], avA[64:65, :])
            nc.vector.tensor_copy(dsB[0:1, :], avB[64:65, :])
            rfA = r_p.tile([1, 512], f32, tag="rfA", name=f"rfA{qc}{pr}")
            rfB = r_p.tile([1, 512], f32, tag="rfB", name=f"rfB{qc}{pr}")
            nc.vector.reciprocal_approx_fast(rfA[0:1, :], dsA[0:1, :])
            nc.vector.reciprocal_approx_fast(rfB[0:1, :], dsB[0:1, :])
            bcsA = r_p.tile([64, 512], f32, tag="bcsA", name=f"bcsA{qc}{pr}")
            bcsB = r_p.tile([64, 512], f32, tag="bcsB", name=f"bcsB{qc}{pr}")
            nc.gpsimd.partition_broadcast(bcsA[0:64, :], rfA[0:1, :], channels=64)
            nc.gpsimd.partition_broadcast(bcsB[0:64, :], rfB[0:1, :], channels=64)
            nc.vector.tensor_mul(
                attnN[pr][0:64, q0 : q0 + 512], avA[0:64, :], bcsA[0:64, :]
            )
            nc.vector.tensor_mul(
                attnN[pr][64:128, q0 : q0 + 512], avB[0:64, :], bcsB[0:64, :]
            )

        for w in windows:
            window(*w)
        # O(1) tail
        for qt in range(4):
            o_unit(1, qt)

    nc.compile()
    return nc


def _prep_inputs(query, key, value, mask, wq_w, wq_b, wk_w, wk_b, wv_w, wv_b, wo_w, wo_b,
                 mm_dtype="bf16"):
    import ml_dtypes

    od = {"bf16": ml_dtypes.bfloat16, "f32r": np.float32, "f32": np.float32}[mm_dtype]
    f = lambda a: np.ascontiguousarray(np.asarray(a, dtype=np.float32))
    g = lambda a: np.ascontiguousarray(np.asarray(a).astype(od))
    query, key, value = f(query), f(key), f(value)
    wq_w, wk_w, wv_w, wo_w = f(wq_w), f(wk_w), f(wv_w), f(wo_w)
    mask = np.asarray(mask)

    keeps = [np.flatnonzero(mask[b] == 0) for b in range(B)]
    cnts = [len(k) for k in keeps]
    assert min(cnts) > 0, "all-masked batch not supported"
    s_pad = max(128, ((max(cnts) + 127) // 128) * 128)
    nsk = s_pad // 128

    bq_pp = np.ascontiguousarray(f(wq_b).reshape(4, 128).T)
    bk_pp = np.ascontiguousarray(f(wk_b).reshape(4, 128).T)
    bvb = np.broadcast_to(f(wv_b).reshape(1, D), (128, D))
    bob = np.broadcast_to(f(wo_b).reshape(1, D), (128, D))

    def wchunks(w):
        # [512, 512] -> [128, 4*512]: col block dk holds rows dk*128..dk*128+128
        return w.reshape(4, 128, D).transpose(1, 0, 2).reshape(128, 4 * D)

    def xchunks(xt):
        # [512, S] -> [128, 4*S]
        s = xt.shape[1]
        return xt.reshape(4, 128, s).transpose(1, 0, 2).reshape(128, 4 * s)

    blob_o = np.concatenate([wchunks(f(wo_w)), bob], axis=1)

    common = dict(
        bq_pp=bq_pp, bk_pp=bk_pp,
        blob_o=g(blob_o),
    )
    in_maps = []
    for b in range(B):
        kc = np.zeros((s_pad, D), np.float32)
        kc[: cnts[b]] = key[b][keeps[b]]
        vc = np.zeros((s_pad, D), np.float32)
        vc[: cnts[b]] = value[b][keeps[b]]
        blob_k = g(np.concatenate([wchunks(f(wk_w)), xchunks(kc.T)], axis=1))
        blob_v = g(
            np.concatenate(
                [
                    wchunks(f(wv_w)),
                    xchunks(vc.T),
                    bvb,
                    np.ones((128, 8), np.float32),
                ],
                axis=1,
            )
        )
        mbf = np.zeros(s_pad, np.float32)
        mbf[cnts[b] :] = -1e9
        mbd = np.ascontiguousarray(mbf.reshape(nsk, 128).T)
        for qh in range(2):
            blob_q = g(
                np.concatenate(
                    [wchunks(f(wq_w)), xchunks(query[b, qh * SQ : (qh + 1) * SQ, :].T)],
                    axis=1,
                )
            )
            in_maps.append(
                dict(blob_k=blob_k, blob_q=blob_q, blob_v=blob_v, mb=mbd, **common)
            )
    return s_pad, in_maps


def kernel(**inputs):
    from concourse import bass_utils

    mmd = os.environ.get("BASSK_MMDT", "bf16")
    s_pad, in_maps = _prep_inputs(**inputs, mm_dtype=mmd)
    key = (s_pad, mmd)
    if key not in _BUILT:
        _BUILT[key] = build_bass(s_pad, mm_dtype=key[1])
    nc = _BUILT[key]
    kw = {}
    if os.environ.get("BASSK_TRACE"):
        kw = dict(trace=True, stitch_traces=False)
    res = bass_utils.run_bass_kernel_spmd(nc, in_maps, core_ids=list(range(NCORE)), **kw)
    out = np.empty((B, S, D), np.float32)
    for c in range(NCORE):
        b, qh = c // 2, c % 2
        out[b, qh * SQ : (qh + 1) * SQ, :] = res.results[c]["out"]
    kernel.last_result = res
    return out


# revision 7
# speedup vs baseline: 1.1357x; 1.1357x over previous
"""Multi-head attention (B=4, S=2048, D=512, H=8) on 8 trn2 NeuronCores.

Sharding: core c = (batch b = c//2, query-half qh = c%2). Each core computes
the full attention output for 1024 query rows of one batch element.

Device-side scheme (all layouts chosen so no on-chip transposes are needed):
  - host supplies transposed activations (X^T = [D, S] layouts)
  - Q^T, K^T projections: lhsT = weight chunk, rhs = X^T chunk; per-partition
    bias added during the PSUM->SBUF drain.
  - V projection in natural [s, dout] layout (lhsT = X^T chunk, rhs = wv),
    stored as [128, H, DH+1] with a ones column per head (denominator trick).
  - logits computed transposed: lg^T[s_k, q] = K_h^T-block . Q_h^T, two heads
    per PE pass (row-packed at partitions 0-63 / 64-127, K=64 each).
  - exp on ACT engine with per-partition mask bias: w = exp(0.125*lg + mb).
    No max-subtraction: logits ~ N(0,1) here, exp is safe in fp32, and
    masked/padded keys get bias -1e9 -> exp exactly 0 (matches reference).
  - AV: attnU^T[d, q] accumulated col-packed (head A -> psum rows 0-63,
    head B -> rows 64-127 of a second tile; concurrent on the PE array)
    + m=1 denominator matmuls against the ones column.
  - normalization: r = 1/denom (DVE), partition-broadcast via K=1 PE outer
    products, two DVE multiplies.
  - O projection: out[q, dout] with lhsT = attnN^T (natural layout), K=1
    matmul adds the output bias row.

Masked keys (mask==1) are compacted away on the host: their softmax weight is
exactly 0 in the reference (exp underflows), so dropping them is exact and
roughly halves attention compute. Keys are padded to a multiple of 128 with
-1e9 mask bias.

Matmul operands are bitcast to float32r (full-rate fp32 path on the PE; plain
fp32 matmul runs at 1/4 rate).
"""

import os
import numpy as np

B, S, D, H = 4, 2048, 512, 8
DH = D // H
NCORE = 8
SQ = S // 2  # queries per core
SCALE = 1.0 / float(np.sqrt(DH))

_BUILT = {}


def _chunks(total, step):
    out = []
    c0 = 0
    while c0 < total:
        out.append((c0, min(step, total - c0)))
        c0 += step
    return out


def build_bass(s_pad, mm_dtype="bf16"):
    import concourse.bass as bass  # noqa: F401
    import concourse.mybir as mybir
    import concourse.tile as tile
    from concourse import bacc
    from contextlib import ExitStack

    f32 = mybir.dt.float32
    mmdt = {
        "bf16": mybir.dt.bfloat16,
        "f32r": mybir.dt.float32r,
        "f32": mybir.dt.float32,
    }[mm_dtype]
    f32r = mybir.dt.float32r
    EXP = mybir.ActivationFunctionType.Exp

    nsk = s_pad // 128

    nc = bacc.Bacc(
        "TRN2",
        target_bir_lowering=False,
        debug=False,
        enable_asserts=False,
        num_devices=NCORE,
    )

    KW, QW = 4 * s_pad, 4 * SQ
    d_bk_blob = nc.dram_tensor("blob_k", [128, 2048 + KW], mmdt, kind="ExternalInput").ap()
    d_bq_blob = nc.dram_tensor("blob_q", [128, 2048 + QW], mmdt, kind="ExternalInput").ap()
    d_bv_blob = nc.dram_tensor("blob_v", [128, 2048 + KW + D + 8], mmdt, kind="ExternalInput").ap()
    d_bo_blob = nc.dram_tensor("blob_o", [128, 2048 + D], mmdt, kind="ExternalInput").ap()
    d_mb = nc.dram_tensor("mb", [128, nsk], f32, kind="ExternalInput").ap()
    d_bq = nc.dram_tensor("bq_pp", [128, 4], f32, kind="ExternalInput").ap()
    d_bk = nc.dram_tensor("bk_pp", [128, 4], f32, kind="ExternalInput").ap()

    def chunked_load(nc, tile_ap, src_ap, width, piece=1024):
        c0 = 0
        while c0 < width:
            cw = min(piece, width - c0)
            nc.sync.dma_start(tile_ap[:, c0 : c0 + cw], src_ap[:, c0 : c0 + cw])
            c0 += cw
    d_out = nc.dram_tensor("out", [SQ, D], f32, kind="ExternalOutput").ap()

    def r(ap):
        return ap

    with tile.TileContext(nc) as tc, ExitStack() as ctx, nc.allow_low_precision(
        "matmul operands held as float32r (full-rate PE fp32 path)"
    ):
        sb = ctx.enter_context(tc.tile_pool(name="sb", bufs=1))
        ps_lg = ctx.enter_context(tc.tile_pool(name="pslg", bufs=2, space="PSUM"))
        ps_av = ctx.enter_context(tc.tile_pool(name="psav", bufs=2, space="PSUM"))

        def load(pool, name, shape, src, dt=None):
            t = pool.tile(shape, mmdt if dt is None else dt, tag=name, name=name)
            nc.sync.dma_start(t[:], src)
            return t

        # projection outputs (persistent)
        kT = [sb.tile([128, s_pad], mmdt, tag=f"kT{j}", name=f"kT{j}") for j in range(4)]
        qT = [sb.tile([128, SQ], mmdt, tag=f"qT{j}", name=f"qT{j}") for j in range(4)]
        attnN = [
            sb.tile([128, SQ], mmdt, tag=f"attnN{pr}", name=f"attnN{pr}")
            for pr in range(4)
        ]
        v = [
            sb.tile([128, H, DH + 1], mmdt, tag=f"v{t}", name=f"v{t}")
            for t in range(nsk)
        ]

        # ---- projection phase (inputs in a scoped pool, freed afterwards) ----
        with tc.tile_pool(name="inp", bufs=1) as inp:
            # chunked loads in consumption order so the PE can start early
            blk = inp.tile([128, 2048 + KW], mmdt, tag="blk", name="blk")
            chunked_load(nc, blk, d_bk_blob, 2048 + KW)
            bk = load(inp, "bk", [128, 4], d_bk[:], dt=f32)
            blq = inp.tile([128, 2048 + QW], mmdt, tag="blq", name="blq")
            chunked_load(nc, blq, d_bq_blob, 2048 + QW)
            bq = load(inp, "bq", [128, 4], d_bq[:], dt=f32)
            blv = inp.tile([128, 2048 + KW + D + 8], mmdt, tag="blv", name="blv")
            chunked_load(nc, blv, d_bv_blob, 2048 + KW + D + 8)
            mb = load(sb, "mb", [128, nsk], d_mb[:], dt=f32)
            blo = sb.tile([128, 2048 + D], mmdt, tag="blo", name="blo")
            chunked_load(nc, blo, d_bo_blob, 2048 + D)
            w_t = {
                "wk": [blk[:, dk * 512 : (dk + 1) * 512] for dk in range(4)],
                "wq": [blq[:, dk * 512 : (dk + 1) * 512] for dk in range(4)],
                "wv": [blv[:, dk * 512 : (dk + 1) * 512] for dk in range(4)],
            }
            xk = [blk[:, 2048 + dk * s_pad : 2048 + (dk + 1) * s_pad] for dk in range(4)]
            xq = [blq[:, 2048 + dk * SQ : 2048 + (dk + 1) * SQ] for dk in range(4)]
            xv = [blv[:, 2048 + dk * s_pad : 2048 + (dk + 1) * s_pad] for dk in range(4)]
            bvb = blv[:, 2048 + KW : 2048 + KW + D]
            ones8 = blv[:, 2048 + KW + D : 2048 + KW + D + 8]
            wo_t = [blo[:, dk * 512 : (dk + 1) * 512] for dk in range(4)]
            bob = blo[:, 2048 : 2048 + D]

            # K^T projection -> kT[j] [128, s_pad] (dout tile j; heads 2j, 2j+1)
            for j in range(4):
                for c0, cw in _chunks(s_pad, 512):
                    ps = ps_lg.tile([128, cw], f32, tag="lg", name="lg")
                    for dk in range(4):
                        nc.tensor.matmul(
                            ps[:],
                            lhsT=r(w_t["wk"][dk][:, j * 128 : (j + 1) * 128]),
                            rhs=r(xk[dk][:, c0 : c0 + cw]),
                            start=(dk == 0),
                            stop=(dk == 3),
                        )
                    nc.vector.tensor_scalar_add(
                        kT[j][:, c0 : c0 + cw], ps[:], bk[:, j : j + 1]
                    )
            # Q^T projection -> qT[j] [128, SQ]
            for j in range(4):
                for c0, cw in _chunks(SQ, 512):
                    ps = ps_lg.tile([128, cw], f32, tag="lg", name="lg")
                    for dk in range(4):
                        nc.tensor.matmul(
                            ps[:],
                            lhsT=r(w_t["wq"][dk][:, j * 128 : (j + 1) * 128]),
                            rhs=r(xq[dk][:, c0 : c0 + cw]),
                            start=(dk == 0),
                            stop=(dk == 3),
                        )
                    nc.vector.tensor_scalar_add(
                        qT[j][:, c0 : c0 + cw], ps[:], bq[:, j : j + 1]
                    )
            # V projection -> v[t] [128, H, DH+1] with ones column
            for t in range(nsk):
                ps = ps_lg.tile([128, D], f32, tag="lg", name="lg")
                for dk in range(4):
                    nc.tensor.matmul(
                        ps[:],
                        lhsT=r(xv[dk][:, t * 128 : (t + 1) * 128]),
                        rhs=r(w_t["wv"][dk]),
                        start=(dk == 0),
                        stop=(dk == 3),
                    )
                nc.vector.tensor_copy(
                    v[t][:, :, DH : DH + 1], ones8.rearrange("p (h o) -> p h o", o=1)
                )
                nc.vector.scalar_tensor_tensor(
                    v[t][:, :, 0:DH],
                    ps[:].rearrange("p (h d) -> p h d", h=H),
                    1.0,
                    bvb.rearrange("p (h d) -> p h d", h=H),
                    op0=mybir.AluOpType.mult,
                    op1=mybir.AluOpType.add,
                )

        # ---- attention phase ----
        wexp_p = ctx.enter_context(tc.tile_pool(name="wexp", bufs=4))
        osb_p = ctx.enter_context(tc.tile_pool(name="osb", bufs=2))
        r_p = ctx.enter_context(tc.tile_pool(name="rp", bufs=2))

        def sk_loop(qc, pr):
            q0 = qc * 512
            hA, hB = 2 * pr, 2 * pr + 1
            avA = ps_av.tile([65, 512], f32, tag="avA", name="avA")
            avB = ps_av.tile([65, 512], f32, tag="avB", name="avB")
            for t in range(nsk):
                lg = ps_lg.tile([128, 1024], f32, tag="lg", name="lg")
                nc.tensor.matmul(
                    lg[:, 0:512],
                    lhsT=r(kT[pr][0:64, t * 128 : (t + 1) * 128]),
                    rhs=r(qT[pr][0:64, q0 : q0 + 512]),
                    start=True,
                    stop=True,
                )
                nc.tensor.matmul(
                    lg[:, 512:1024],
                    lhsT=r(kT[pr][64:128, t * 128 : (t + 1) * 128]),
                    rhs=r(qT[pr][64:128, q0 : q0 + 512]),
                    start=True,
                    stop=True,
                )
                wx = wexp_p.tile([128, 1024], mmdt, tag="wexp", name="wexp")
                nc.scalar.activation(
                    wx[:], lg[:], EXP, bias=mb[:, t : t + 1], scale=SCALE
                )
                last = t == nsk - 1
                nc.tensor.matmul(
                    avA[0:65, :],
                    lhsT=r(v[t][:, hA : hA + 1, 0 : DH + 1]),
                    rhs=r(wx[:, 0:512]),
                    start=(t == 0),
                    stop=last,
                )
                nc.tensor.matmul(
                    avB[0:65, :],
                    lhsT=r(v[t][:, hB : hB + 1, 0 : DH + 1]),
                    rhs=r(wx[:, 512:1024]),
                    start=(t == 0),
                    stop=last,
                )
            # kick off the DVE reciprocal chain now; bc matmuls are emitted
            # later (pipelined) so the PE never waits on this chain
            dsA = r_p.tile([1, 512], f32, tag="dsA", name="dsA")
            dsB = r_p.tile([1, 512], f32, tag="dsB", name="dsB")
            nc.vector.tensor_copy(dsA[0:1, :], avA[64:65, :])
            nc.vector.tensor_copy(dsB[0:1, :], avB[64:65, :])
            rfA = r_p.tile([1, 512], f32, tag="rfA", name="rfA")
            rfB = r_p.tile([1, 512], f32, tag="rfB", name="rfB")
            nc.vector.reciprocal_approx_fast(rfA[0:1, :], dsA[0:1, :])
            nc.vector.reciprocal_approx_fast(rfB[0:1, :], dsB[0:1, :])
            bcsA = r_p.tile([64, 512], f32, tag="bcsA", name="bcsA")
            bcsB = r_p.tile([64, 512], f32, tag="bcsB", name="bcsB")
            nc.gpsimd.partition_broadcast(bcsA[0:64, :], rfA[0:1, :], channels=64)
            nc.gpsimd.partition_broadcast(bcsB[0:64, :], rfB[0:1, :], channels=64)
            return (qc, pr, avA, avB, bcsA, bcsB)

        def finish(qc, pr, avA, avB, bcsA, bcsB):
            q0 = qc * 512
            nc.vector.tensor_mul(
                attnN[pr][0:64, q0 : q0 + 512], avA[0:64, :], bcsA[0:64, :]
            )
            nc.vector.tensor_mul(
                attnN[pr][64:128, q0 : q0 + 512], avB[0:64, :], bcsB[0:64, :]
            )

        def o_proj(qc):
            q0 = qc * 512
            for qt in range(4):
                qq = q0 + qt * 128
                ops = ps_lg.tile([128, D], f32, tag="lg", name="ops")
                for pr2 in range(4):
                    nc.tensor.matmul(
                        ops[:],
                        lhsT=r(attnN[pr2][:, qq : qq + 128]),
                        rhs=r(wo_t[pr2]),
                        start=(pr2 == 0),
                        stop=(pr2 == 3),
                    )
                osb = osb_p.tile([128, D], f32, tag="osb", name="osb")
                nc.vector.scalar_tensor_tensor(
                    osb[:],
                    ops[:],
                    1.0,
                    bob,
                    op0=mybir.AluOpType.mult,
                    op1=mybir.AluOpType.add,
                )
                nc.sync.dma_start(d_out[qq : qq + 128, :], osb[:])

        pend = None
        oproj_due = None
        for qc in range(SQ // 512):
            for pr in range(4):
                st = sk_loop(qc, pr)
                if pend is not None:
                    finish(*pend)
                    pend = None
                    if pr == 0 and oproj_due is not None:
                        o_proj(oproj_due)
                        oproj_due = None
                pend = st
            oproj_due = qc
        finish(*pend)
        o_proj(oproj_due)

    nc.compile()
    return nc


def _prep_inputs(query, key, value, mask, wq_w, wq_b, wk_w, wk_b, wv_w, wv_b, wo_w, wo_b,
                 mm_dtype="bf16"):
    import ml_dtypes

    od = {"bf16": ml_dtypes.bfloat16, "f32r": np.float32, "f32": np.float32}[mm_dtype]
    f = lambda a: np.ascontiguousarray(np.asarray(a, dtype=np.float32))
    g = lambda a: np.ascontiguousarray(np.asarray(a).astype(od))
    query, key, value = f(query), f(key), f(value)
    wq_w, wk_w, wv_w, wo_w = f(wq_w), f(wk_w), f(wv_w), f(wo_w)
    mask = np.asarray(mask)

    keeps = [np.flatnonzero(mask[b] == 0) for b in range(B)]
    cnts = [len(k) for k in keeps]
    assert min(cnts) > 0, "all-masked batch not supported"
    s_pad = max(128, ((max(cnts) + 127) // 128) * 128)
    nsk = s_pad // 128

    bq_pp = np.ascontiguousarray(f(wq_b).reshape(4, 128).T)
    bk_pp = np.ascontiguousarray(f(wk_b).reshape(4, 128).T)
    bvb = np.broadcast_to(f(wv_b).reshape(1, D), (128, D))
    bob = np.broadcast_to(f(wo_b).reshape(1, D), (128, D))

    def wchunks(w):
        # [512, 512] -> [128, 4*512]: col block dk holds rows dk*128..dk*128+128
        return w.reshape(4, 128, D).transpose(1, 0, 2).reshape(128, 4 * D)

    def xchunks(xt):
        # [512, S] -> [128, 4*S]
        s = xt.shape[1]
        return xt.reshape(4, 128, s).transpose(1, 0, 2).reshape(128, 4 * s)

    blob_o = np.concatenate([wchunks(f(wo_w)), bob], axis=1)

    common = dict(
        bq_pp=bq_pp, bk_pp=bk_pp,
        blob_o=g(blob_o),
    )
    in_maps = []
    for b in range(B):
        kc = np.zeros((s_pad, D), np.float32)
        kc[: cnts[b]] = key[b][keeps[b]]
        vc = np.zeros((s_pad, D), np.float32)
        vc[: cnts[b]] = value[b][keeps[b]]
        blob_k = g(np.concatenate([wchunks(f(wk_w)), xchunks(kc.T)], axis=1))
        blob_v = g(
            np.concatenate(
                [
                    wchunks(f(wv_w)),
                    xchunks(vc.T),
                    bvb,
                    np.ones((128, 8), np.float32),
                ],
                axis=1,
            )
        )
        mbf = np.zeros(s_pad, np.float32)
        mbf[cnts[b] :] = -1e9
        mbd = np.ascontiguousarray(mbf.reshape(nsk, 128).T)
        for qh in range(2):
            blob_q = g(
                np.concatenate(
                    [wchunks(f(wq_w)), xchunks(query[b, qh * SQ : (qh + 1) * SQ, :].T)],
                    axis=1,
                )
            )
            in_maps.append(
                dict(blob_k=blob_k, blob_q=blob_q, blob_v=blob_v, mb=mbd, **common)
            )
    return s_pad, in_maps


def kernel(**inputs):
    from concourse import bass_utils

    mmd = os.environ.get("BASSK_MMDT", "bf16")
    s_pad, in_maps = _prep_inputs(**inputs, mm_dtype=mmd)
    key = (s_pad, mmd)
    if key not in _BUILT:
        _BUILT[key] = build_bass(s_pad, mm_dtype=key[1])
    nc = _BUILT[key]
    kw = {}
    if os.environ.get("BASSK_TRACE"):
        kw = dict(trace=True, stitch_traces=False)
    res = bass_utils.run_bass_kernel_spmd(nc, in_maps, core_ids=list(range(NCORE)), **kw)
    out = np.empty((B, S, D), np.float32)
    for c in range(NCORE):
        b, qh = c // 2, c % 2
        out[b, qh * SQ : (qh + 1) * SQ, :] = res.results[c]["out"]
    kernel.last_result = res
    return out



# revision 12
# speedup vs baseline: 1.1953x; 1.0525x over previous
"""Multi-head attention (B=4, S=2048, D=512, H=8) on 8 trn2 NeuronCores.

Sharding: core c = (batch b = c//2, query-half qh = c%2). Each core computes
the full attention output for 1024 query rows of one batch element.

v2.1: single fully-pipelined phase. The attention inner loop is ACT-bound
(exp of [128,1024] logit tiles, 72 of them); projection and O-projection
matmuls are injected into the PE stream inside the windows, filling the PE's
exp-wait slack. Window w+1 uses the opposite PSUM av tag-set from window w,
so w's normalization chain (reciprocal -> partition broadcast -> multiply)
never blocks w+1's AV matmuls. Injected units borrow the av tag-set not used
by the current window (slots >= 3 only, after the previous window's release).
The last window pre-accumulates half the O-projection and normalizes via a
K=1 PE broadcast to shorten the tail. DMA pieces are issued in consumption
order; warm-up matmuls keep HAM un-throttled from ~6us.

Device-side scheme (unchanged):
  - host supplies transposed activations (X^T layouts); Q^T/K^T projections
    with per-partition bias in the drain; V in natural layout [128, H, DH+1]
    with a ones column (softmax denominator rides in the AV matmul, M=65).
  - logits transposed: lg^T[s_k, q], two heads per PE pass (row-packed K=64).
  - exp on ACT: w = exp(0.125*lg + mb); masked/padded keys get bias -1e9.
  - AV accumulated per head over key blocks; normalization via reciprocal +
    partition broadcast; O projection streamed to HBM per 128 rows.

Masked keys (mask==1) are compacted away on the host (exact), keys padded to
a multiple of 128 with -1e9 mask bias.
"""

import os
import numpy as np

B, S, D, H = 4, 2048, 512, 8
DH = D // H
NCORE = 8
SQ = S // 2  # queries per core
SCALE = 1.0 / float(np.sqrt(DH))

_BUILT = {}


def _chunks(total, step):
    out = []
    c0 = 0
    while c0 < total:
        out.append((c0, min(step, total - c0)))
        c0 += step
    return out


def build_bass(s_pad, mm_dtype="bf16"):
    import concourse.bass as bass  # noqa: F401
    import concourse.mybir as mybir
    import concourse.tile as tile
    from concourse import bacc
    from contextlib import ExitStack

    f32 = mybir.dt.float32
    mmdt = {
        "bf16": mybir.dt.bfloat16,
        "f32r": mybir.dt.float32r,
        "f32": mybir.dt.float32,
    }[mm_dtype]
    EXP = mybir.ActivationFunctionType.Exp

    nsk = s_pad // 128

    nc = bacc.Bacc(
        "TRN2",
        target_bir_lowering=False,
        debug=False,
        enable_asserts=False,
        num_devices=NCORE,
    )

    KW, QW = 4 * s_pad, 4 * SQ
    d_bk_blob = nc.dram_tensor("blob_k", [128, 2048 + KW], mmdt, kind="ExternalInput").ap()
    d_bq_blob = nc.dram_tensor("blob_q", [128, 2048 + QW], mmdt, kind="ExternalInput").ap()
    d_bv_blob = nc.dram_tensor("blob_v", [128, 2048 + KW + D + 8], mmdt, kind="ExternalInput").ap()
    d_bo_blob = nc.dram_tensor("blob_o", [128, 2048 + D], mmdt, kind="ExternalInput").ap()
    d_mb = nc.dram_tensor("mb", [128, nsk], f32, kind="ExternalInput").ap()
    d_bq = nc.dram_tensor("bq_pp", [128, 4], f32, kind="ExternalInput").ap()
    d_bk = nc.dram_tensor("bk_pp", [128, 4], f32, kind="ExternalInput").ap()
    d_out = nc.dram_tensor("out", [SQ, D], f32, kind="ExternalOutput").ap()

    with tile.TileContext(nc) as tc, ExitStack() as ctx, nc.allow_low_precision(
        "matmul operands held in bf16 (tolerance 2e-2; measured ~6e-3)"
    ):
        sb = ctx.enter_context(tc.tile_pool(name="sb", bufs=1))
        # PSUM budget (16KB/partition): lg 2x[128,1024]f32 = 8KB;
        # av sets 0/1 x (A,B) 1 buf x [128,512]f32 = 8KB. Injected units
        # borrow slots from the av set the current window is NOT using.
        ps_lg = ctx.enter_context(tc.tile_pool(name="pslg", bufs=2, space="PSUM"))
        ps_av = ctx.enter_context(tc.tile_pool(name="psav", bufs=1, space="PSUM"))
        wexp_p = ctx.enter_context(tc.tile_pool(name="wexp", bufs=4))
        osb_p = ctx.enter_context(tc.tile_pool(name="osb", bufs=2))
        r_p = ctx.enter_context(tc.tile_pool(name="rp", bufs=2))

        # ---- persistent SBUF tiles ----
        blk = sb.tile([128, 2048 + KW], mmdt, tag="blk", name="blk")
        blq = sb.tile([128, 2048 + QW], mmdt, tag="blq", name="blq")
        blv = sb.tile([128, 2048 + KW + D + 8], mmdt, tag="blv", name="blv")
        blo = sb.tile([128, 2048 + D], mmdt, tag="blo", name="blo")
        bk = sb.tile([128, 4], f32, tag="bk", name="bk")
        bq = sb.tile([128, 4], f32, tag="bq", name="bq")
        mb = sb.tile([128, nsk], f32, tag="mb", name="mb")
        kT = [sb.tile([128, s_pad], mmdt, tag=f"kT{j}", name=f"kT{j}") for j in range(4)]
        qT = [sb.tile([128, SQ], mmdt, tag=f"qT{j}", name=f"qT{j}") for j in range(4)]
        attnN = [
            sb.tile([128, SQ], mmdt, tag=f"attnN{pr}", name=f"attnN{pr}")
            for pr in range(4)
        ]
        v = [
            sb.tile([128, H, DH + 1], mmdt, tag=f"v{t}", name=f"v{t}")
            for t in range(nsk)
        ]
        wu = sb.tile([128, 512], mmdt, tag="wu", name="wu")

        bvb = blv[:, 2048 + KW : 2048 + KW + D]
        ones8 = blv[:, 2048 + KW + D : 2048 + KW + D + 8]
        bob = blo[:, 2048 : 2048 + D]

        # ---- DMA emission, consumption order ----
        X = 2048  # x-region offset inside k/q/v blobs
        h_k = min(((s_pad // 2 + 127) // 128) * 128, s_pad)
        p_v0 = min(5 * 128, s_pad)  # xv piece 0 covers t=0..4

        def ld(dst, src):
            nc.sync.dma_start(dst, src)

        ld(bk[:], d_bk[:])
        ld(bq[:], d_bq[:])
        ld(mb[:], d_mb[:])
        for c0, cw in _chunks(2048, 1024):  # wk
            ld(blk[:, c0 : c0 + cw], d_bk_blob[:, c0 : c0 + cw])
        for c0, cw in _chunks(2048, 1024):  # wq
            ld(blq[:, c0 : c0 + cw], d_bq_blob[:, c0 : c0 + cw])
        for dk in range(4):  # xk first halves
            o = X + dk * s_pad
            ld(blk[:, o : o + h_k], d_bk_blob[:, o : o + h_k])
        for c0, cw in _chunks(2048, 1024):  # wv
            ld(blv[:, c0 : c0 + cw], d_bv_blob[:, c0 : c0 + cw])
        ld(blv[:, X + KW :], d_bv_blob[:, X + KW :])  # v bias + ones
        for dk in range(4):  # xv piece 0 (t=0..4)
            o = X + dk * s_pad
            ld(blv[:, o : o + p_v0], d_bv_blob[:, o : o + p_v0])
        for dk in range(4):  # xq first halves
            o = X + dk * SQ
            ld(blq[:, o : o + 512], d_bq_blob[:, o : o + 512])
        if h_k < s_pad:  # xk second halves
            for dk in range(4):
                o = X + dk * s_pad + h_k
                e = X + dk * s_pad + s_pad
                ld(blk[:, o:e], d_bk_blob[:, o:e])
        if p_v0 < s_pad:  # xv piece 1
            for dk in range(4):
                o = X + dk * s_pad + p_v0
                e = X + dk * s_pad + s_pad
                ld(blv[:, o:e], d_bv_blob[:, o:e])
        for c0, cw in _chunks(2048 + D, 1024):  # wo + output bias
            ld(blo[:, c0 : c0 + cw], d_bo_blob[:, c0 : c0 + cw])
        for dk in range(4):  # xq second halves (needed ~55us in)
            o = X + dk * SQ + 512
            ld(blq[:, o : o + 512], d_bq_blob[:, o : o + 512])

        # ---- borrowed-psum allocator for injected units ----
        bctx = {"set": 1, "tog": 0, "n": 0}

        def unit_ps(shape=None):
            tag = f"av{bctx['set']}{'AB'[bctx['tog']]}"
            bctx["tog"] ^= 1
            bctx["n"] += 1
            return ps_av.tile(
                shape or [128, 512], f32, tag=tag, name=f"ups{bctx['n']}"
            )

        # ---- PE warm-up (no data deps; runs during preamble/DMA) ----
        nc.vector.memset(wu[:], 0.0)
        for i in range(14):
            wps = unit_ps()
            nc.tensor.matmul(wps[:], lhsT=wu[:, 0:128], rhs=wu[:], start=True, stop=True)

        # ---- projection emitters ----
        def kq_chunk(which, j, c0, cw):
            blob, bias_t, dst = (blk, bk, kT) if which == "k" else (blq, bq, qT)
            sp = s_pad if which == "k" else SQ
            ps = unit_ps()
            for dk in range(4):
                nc.tensor.matmul(
                    ps[:, 0:cw],
                    lhsT=blob[:, dk * 512 + j * 128 : dk * 512 + (j + 1) * 128],
                    rhs=blob[:, X + dk * sp + c0 : X + dk * sp + c0 + cw],
                    start=(dk == 0),
                    stop=(dk == 3),
                )
            nc.vector.tensor_scalar_add(
                dst[j][:, c0 : c0 + cw], ps[:, 0:cw], bias_t[:, j : j + 1]
            )

        def v_proj(t):
            ps = unit_ps()
            for dk in range(4):
                nc.tensor.matmul(
                    ps[:],
                    lhsT=blv[:, X + dk * s_pad + t * 128 : X + dk * s_pad + (t + 1) * 128],
                    rhs=blv[:, dk * 512 : (dk + 1) * 512],
                    start=(dk == 0),
                    stop=(dk == 3),
                )
            nc.vector.tensor_copy(
                v[t][:, :, DH : DH + 1], ones8.rearrange("p (h o) -> p h o", o=1)
            )
            nc.vector.scalar_tensor_tensor(
                v[t][:, :, 0:DH],
                ps[:].rearrange("p (h d) -> p h d", h=H),
                1.0,
                bvb.rearrange("p (h d) -> p h d", h=H),
                op0=mybir.AluOpType.mult,
                op1=mybir.AluOpType.add,
            )

        def o_drain(qc, qt, ops):
            qq = qc * 512 + qt * 128
            osb = osb_p.tile([128, D], f32, tag="osb", name=f"osb{qc}_{qt}")
            nc.vector.scalar_tensor_tensor(
                osb[:],
                ops[:],
                1.0,
                bob,
                op0=mybir.AluOpType.mult,
                op1=mybir.AluOpType.add,
            )
            nc.sync.dma_start(d_out[qq : qq + 128, :], osb[:])

        def o_unit(qc, qt, ps=None, prs=range(4), stop=True):
            qq = qc * 512 + qt * 128
            if ps is None:
                ps = unit_ps()
            for pr2 in prs:
                nc.tensor.matmul(
                    ps[:],
                    lhsT=attnN[pr2][:, qq : qq + 128],
                    rhs=blo[:, pr2 * 512 : (pr2 + 1) * 512],
                    start=(pr2 == 0),
                    stop=(pr2 == 3 and stop),
                )
            if stop:
                o_drain(qc, qt, ps)
            return ps

        # ---- up-front projections (in DMA-arrival order) ----
        kch = _chunks(s_pad, 512)
        kq_chunk("k", 0, 0, 512)
        for t in range(min(5, nsk)):
            v_proj(t)
        kq_chunk("q", 0, 0, 512)

        # ---- injection schedule ----
        main_q = []
        for c0, cw in kch[1:]:
            main_q.append(("k", 0, c0, cw))
        for t in range(5, nsk):
            main_q.append(("v", t))
        for j in (1, 2, 3):
            main_q.append(("q", j, 0, 512))
            for c0, cw in kch:
                main_q.append(("k", j, c0, cw))
        main_q.append(("q", 0, 512, 512))
        for j in (1, 2, 3):
            main_q.append(("q", j, 512, 512))

        def run_unit(u):
            if u[0] == "v":
                v_proj(u[1])
            elif u[0] in ("k", "q"):
                kq_chunk(u[0], u[1], u[2], u[3])
            elif u[0] == "o":
                o_unit(u[1], u[2])
            elif u[0] == "o1p":  # O(1) partial pre-accumulation (pr 0..2)
                o1_ps[u[2]] = o_unit(u[1], u[2], prs=range(3), stop=False)

        windows = [(qc, pr) for qc in range(2) for pr in range(4)]
        slot_plan = {w: [[] for _ in range(nsk)] for w in windows}
        # units only in slots >= 3 (borrowed av set releases ~1/3 into window)
        cap = {w: [0, 0, 0] + [1] * (nsk - 3) for w in windows}
        for sl in range(max(3, nsk - 2), nsk):
            cap[(0, 0)][sl] = 2
        for qt in range(4):  # O(0) in window (1,0)
            sl = min(3 + qt, nsk - 1)
            slot_plan[(1, 0)][sl].append(("o", 0, qt))
            cap[(1, 0)][sl] = 0
        for qt in (0, 1):  # O(1) qt0/qt1 partials in window (1,3)
            sl = min(3 + qt, nsk - 1)
            slot_plan[(1, 3)][sl].append(("o1p", 1, qt))
            cap[(1, 3)][sl] = 0

        def unit_ns(u):
            if u[0] in ("k", "q"):
                return 860 if u[3] >= 512 else 290
            return 900

        budget = {w: (99000 if w == (0, 0) else 5000) for w in windows}
        fill = []
        for w in windows:
            for sl in range(nsk):
                for _ in range(cap[w][sl]):
                    fill.append((w, sl))
        pos = 0
        for u in main_q:
            while pos < len(fill):
                w, sl = fill[pos]
                if budget[w] >= unit_ns(u):
                    break
                while pos < len(fill) and fill[pos][0] == w:
                    pos += 1
            if pos < len(fill):
                w, sl = fill[pos]
                slot_plan[w][sl].append(u)
                budget[w] -= unit_ns(u)
                pos += 1
            else:
                slot_plan[windows[-1]].append([u])

        o1_ps = {}

        # ---- attention windows ----
        def window(wi, qc, pr):
            q0 = qc * 512
            hA, hB = 2 * pr, 2 * pr + 1
            st = wi % 2
            bctx["set"] = 1 - st  # units borrow the other av set
            avA = ps_av.tile([65, 512], f32, tag=f"av{st}A", name=f"avA{qc}{pr}")
            avB = ps_av.tile([65, 512], f32, tag=f"av{st}B", name=f"avB{qc}{pr}")
            plan = slot_plan[(qc, pr)]
            pend = None

            def emit_av(t, wx):
                last = t == nsk - 1
                nc.tensor.matmul(
                    avA[0:65, :],
                    lhsT=v[t][:, hA : hA + 1, 0 : DH + 1],
                    rhs=wx[:, 0:512],
                    start=(t == 0),
                    stop=last,
                )
                nc.tensor.matmul(
                    avB[0:65, :],
                    lhsT=v[t][:, hB : hB + 1, 0 : DH + 1],
                    rhs=wx[:, 512:1024],
                    start=(t == 0),
                    stop=last,
                )

            for t in range(nsk):
                lg = ps_lg.tile([128, 1024], f32, tag="lg", name=f"lg{qc}{pr}_{t}")
                nc.tensor.matmul(
                    lg[:, 0:512],
                    lhsT=kT[pr][0:64, t * 128 : (t + 1) * 128],
                    rhs=qT[pr][0:64, q0 : q0 + 512],
                    start=True,
                    stop=True,
                )
                nc.tensor.matmul(
                    lg[:, 512:1024],
                    lhsT=kT[pr][64:128, t * 128 : (t + 1) * 128],
                    rhs=qT[pr][64:128, q0 : q0 + 512],
                    start=True,
                    stop=True,
                )
                wx = wexp_p.tile([128, 1024], mmdt, tag="wexp", name=f"wx{qc}{pr}_{t}")
                nc.scalar.activation(wx[:], lg[:], EXP, bias=mb[:, t : t + 1], scale=SCALE)
                if pend is not None:
                    emit_av(*pend)
                for u in plan[t]:
                    run_unit(u)
                pend = (t, wx)
            emit_av(*pend)
            for lst in plan[nsk:]:
                for u in lst:
                    run_unit(u)

            # normalization (reciprocal needs SBUF input: copy denom rows out)
            dsA = r_p.tile([1, 512], f32, tag="dsA", name=f"dsA{qc}{pr}")
            dsB = r_p.tile([1, 512], f32, tag="dsB", name=f"dsB{qc}{pr}")
            nc.vector.tensor_copy(dsA[0:1, :], avA[64:65, :])
            nc.vector.tensor_copy(dsB[0:1, :], avB[64:65, :])
            rfA = r_p.tile([1, 512], f32, tag="rfA", name=f"rfA{qc}{pr}")
            rfB = r_p.tile([1, 512], f32, tag="rfB", name=f"rfB{qc}{pr}")
            nc.vector.reciprocal_approx_fast(rfA[0:1, :], dsA[0:1, :])
            nc.vector.reciprocal_approx_fast(rfB[0:1, :], dsB[0:1, :])
            bcsA = r_p.tile([64, 512], f32, tag="bcsA", name=f"bcsA{qc}{pr}")
            bcsB = r_p.tile([64, 512], f32, tag="bcsB", name=f"bcsB{qc}{pr}")
            nc.gpsimd.partition_broadcast(bcsA[0:64, :], rfA[0:1, :], channels=64)
            nc.gpsimd.partition_broadcast(bcsB[0:64, :], rfB[0:1, :], channels=64)
            nc.vector.tensor_mul(
                attnN[pr][0:64, q0 : q0 + 512], avA[0:64, :], bcsA[0:64, :]
            )
            nc.vector.tensor_mul(
                attnN[pr][64:128, q0 : q0 + 512], avB[0:64, :], bcsB[0:64, :]
            )

        for wi, w in enumerate(windows):
            window(wi, *w)

        # ---- tail: finalize O(1) ----
        for qt in (0, 1):  # pre-accumulated: one matmul (pr=3) + drain
            ps = o1_ps[qt]
            qq = 512 + qt * 128
            nc.tensor.matmul(
                ps[:],
                lhsT=attnN[3][:, qq : qq + 128],
                rhs=blo[:, 3 * 512 : 4 * 512],
                start=False,
                stop=True,
            )
            o_drain(1, qt, ps)
        for qt in (2, 3):  # full O units on now-free lg slots
            ps = ps_lg.tile([128, 1024], f32, tag="lg", name=f"o1f{qt}")[:, 0:512]
            o_unit(1, qt, ps=ps)

    nc.compile()
    return nc


def _prep_inputs(query, key, value, mask, wq_w, wq_b, wk_w, wk_b, wv_w, wv_b, wo_w, wo_b,
                 mm_dtype="bf16"):
    import ml_dtypes

    od = {"bf16": ml_dtypes.bfloat16, "f32r": np.float32, "f32": np.float32}[mm_dtype]
    f = lambda a: np.ascontiguousarray(np.asarray(a, dtype=np.float32))
    g = lambda a: np.ascontiguousarray(np.asarray(a).astype(od))
    query, key, value = f(query), f(key), f(value)
    wq_w, wk_w, wv_w, wo_w = f(wq_w), f(wk_w), f(wv_w), f(wo_w)
    mask = np.asarray(mask)

    keeps = [np.flatnonzero(mask[b] == 0) for b in range(B)]
    cnts = [len(k) for k in keeps]
    assert min(cnts) > 0, "all-masked batch not supported"
    s_pad = max(128, ((max(cnts) + 127) // 128) * 128)
    nsk = s_pad // 128

    bq_pp = np.ascontiguousarray(f(wq_b).reshape(4, 128).T)
    bk_pp = np.ascontiguousarray(f(wk_b).reshape(4, 128).T)
    bvb = np.broadcast_to(f(wv_b).reshape(1, D), (128, D))
    bob = np.broadcast_to(f(wo_b).reshape(1, D), (128, D))

    def wchunks(w):
        # [512, 512] -> [128, 4*512]: col block dk holds rows dk*128..dk*128+128
        return w.reshape(4, 128, D).transpose(1, 0, 2).reshape(128, 4 * D)

    def xchunks(xt):
        # [512, S] -> [128, 4*S]
        s = xt.shape[1]
        return xt.reshape(4, 128, s).transpose(1, 0, 2).reshape(128, 4 * s)

    blob_o = np.concatenate([wchunks(f(wo_w)), bob], axis=1)

    common = dict(
        bq_pp=bq_pp, bk_pp=bk_pp,
        blob_o=g(blob_o),
    )
    in_maps = []
    for b in range(B):
        kc = np.zeros((s_pad, D), np.float32)
        kc[: cnts[b]] = key[b][keeps[b]]
        vc = np.zeros((s_pad, D), np.float32)
        vc[: cnts[b]] = value[b][keeps[b]]
        blob_k = g(np.concatenate([wchunks(f(wk_w)), xchunks(kc.T)], axis=1))
        blob_v = g(
            np.concatenate(
                [
                    wchunks(f(wv_w)),
                    xchunks(vc.T),
                    bvb,
                    np.ones((128, 8), np.float32),
                ],
                axis=1,
            )
        )
        mbf = np.zeros(s_pad, np.float32)
        mbf[cnts[b] :] = -1e9
        mbd = np.ascontiguousarray(mbf.reshape(nsk, 128).T)
        for qh in range(2):
            blob_q = g(
                np.concatenate(
                    [wchunks(f(wq_w)), xchunks(query[b, qh * SQ : (qh + 1) * SQ, :].T)],
                    axis=1,
                )
            )
            in_maps.append(
                dict(blob_k=blob_k, blob_q=blob_q, blob_v=blob_v, mb=mbd, **common)
            )
    return s_pad, in_maps


def kernel(**inputs):
    from concourse import bass_utils

    mmd = os.environ.get("BASSK_MMDT", "bf16")
    s_pad, in_maps = _prep_inputs(**inputs, mm_dtype=mmd)
    key = (s_pad, mmd)
    if key not in _BUILT:
        _BUILT[key] = build_bass(s_pad, mm_dtype=key[1])
    nc = _BUILT[key]
    kw = {}
    if os.environ.get("BASSK_TRACE"):
        kw = dict(trace=True, stitch_traces=False)
    res = bass_utils.run_bass_kernel_spmd(nc, in_maps, core_ids=list(range(NCORE)), **kw)
    out = np.empty((B, S, D), np.float32)
    for c in range(NCORE):
        b, qh = c // 2, c % 2
        out[b, qh * SQ : (qh + 1) * SQ, :] = res.results[c]["out"]
    kernel.last_result = res
    return out


# revision 16
# speedup vs baseline: 1.2222x; 1.0225x over previous
"""Multi-head attention (B=4, S=2048, D=512, H=8) on 8 trn2 NeuronCores.

Sharding: core c = (batch b = c//2, query-half qh = c%2). Each core computes
the full attention output for 1024 query rows of one batch element.

v2.1: single fully-pipelined phase. The attention inner loop is ACT-bound
(exp of [128,1024] logit tiles, 72 of them); projection and O-projection
matmuls are injected into the PE stream inside the windows, filling the PE's
exp-wait slack. Window w+1 uses the opposite PSUM av tag-set from window w,
so w's normalization chain (reciprocal -> partition broadcast -> multiply)
never blocks w+1's AV matmuls. Injected units borrow the av tag-set not used
by the current window (slots >= 3 only, after the previous window's release).
The last window pre-accumulates half the O-projection and normalizes via a
K=1 PE broadcast to shorten the tail. DMA pieces are issued in consumption
order; warm-up matmuls keep HAM un-throttled from ~6us.

Device-side scheme (unchanged):
  - host supplies transposed activations (X^T layouts); Q^T/K^T projections
    with per-partition bias in the drain; V in natural layout [128, H, DH+1]
    with a ones column (softmax denominator rides in the AV matmul, M=65).
  - logits transposed: lg^T[s_k, q], two heads per PE pass (row-packed K=64).
  - exp on ACT: w = exp(0.125*lg + mb); masked/padded keys get bias -1e9.
  - AV accumulated per head over key blocks; normalization via reciprocal +
    partition broadcast; O projection streamed to HBM per 128 rows.

Masked keys (mask==1) are compacted away on the host (exact), keys padded to
a multiple of 128 with -1e9 mask bias.
"""

import os
import numpy as np

B, S, D, H = 4, 2048, 512, 8
DH = D // H
NCORE = 8
SQ = S // 2  # queries per core
SCALE = 1.0 / float(np.sqrt(DH))

_BUILT = {}


def _chunks(total, step):
    out = []
    c0 = 0
    while c0 < total:
        out.append((c0, min(step, total - c0)))
        c0 += step
    return out


def build_bass(s_pad, mm_dtype="bf16"):
    import concourse.bass as bass  # noqa: F401
    import concourse.mybir as mybir
    import concourse.tile as tile
    from concourse import bacc
    from contextlib import ExitStack

    f32 = mybir.dt.float32
    mmdt = {
        "bf16": mybir.dt.bfloat16,
        "f32r": mybir.dt.float32r,
        "f32": mybir.dt.float32,
    }[mm_dtype]
    EXP = mybir.ActivationFunctionType.Exp

    nsk = s_pad // 128

    nc = bacc.Bacc(
        "TRN2",
        target_bir_lowering=False,
        debug=False,
        enable_asserts=False,
        num_devices=NCORE,
    )

    KW, QW = 4 * s_pad, 4 * SQ
    d_bk_blob = nc.dram_tensor("blob_k", [128, 2048 + KW], mmdt, kind="ExternalInput").ap()
    d_bq_blob = nc.dram_tensor("blob_q", [128, 2048 + QW], mmdt, kind="ExternalInput").ap()
    d_bv_blob = nc.dram_tensor("blob_v", [128, 2048 + KW + D + 8], mmdt, kind="ExternalInput").ap()
    d_bo_blob = nc.dram_tensor("blob_o", [128, 2048 + D], mmdt, kind="ExternalInput").ap()
    d_mb = nc.dram_tensor("mb", [128, nsk], f32, kind="ExternalInput").ap()
    d_bq = nc.dram_tensor("bq_pp", [128, 4], f32, kind="ExternalInput").ap()
    d_bk = nc.dram_tensor("bk_pp", [128, 4], f32, kind="ExternalInput").ap()
    d_out = nc.dram_tensor("out", [SQ, D], f32, kind="ExternalOutput").ap()

    with tile.TileContext(nc) as tc, ExitStack() as ctx, nc.allow_low_precision(
        "matmul operands held in bf16 (tolerance 2e-2; measured ~6e-3)"
    ):
        sb = ctx.enter_context(tc.tile_pool(name="sb", bufs=1))
        # PSUM budget (16KB/partition): lg 2x[128,1024]f32 = 8KB;
        # av sets 0/1 x (A,B) 1 buf x [128,512]f32 = 8KB. Injected units
        # borrow slots from the av set the current window is NOT using.
        ps_lg = ctx.enter_context(tc.tile_pool(name="pslg", bufs=2, space="PSUM"))
        ps_av = ctx.enter_context(tc.tile_pool(name="psav", bufs=1, space="PSUM"))
        wexp_p = ctx.enter_context(tc.tile_pool(name="wexp", bufs=4))
        osb_p = ctx.enter_context(tc.tile_pool(name="osb", bufs=2))
        r_p = ctx.enter_context(tc.tile_pool(name="rp", bufs=2))

        # ---- persistent SBUF tiles ----
        blk = sb.tile([128, 2048 + KW], mmdt, tag="blk", name="blk")
        blq = sb.tile([128, 2048 + QW], mmdt, tag="blq", name="blq")
        blv = sb.tile([128, 2048 + KW + D + 8], mmdt, tag="blv", name="blv")
        blo = sb.tile([128, 2048 + D], mmdt, tag="blo", name="blo")
        bk = sb.tile([128, 4], f32, tag="bk", name="bk")
        bq = sb.tile([128, 4], f32, tag="bq", name="bq")
        mb = sb.tile([128, nsk], f32, tag="mb", name="mb")
        kT = [sb.tile([128, s_pad], mmdt, tag=f"kT{j}", name=f"kT{j}") for j in range(4)]
        qT = [sb.tile([128, SQ], mmdt, tag=f"qT{j}", name=f"qT{j}") for j in range(4)]
        attnN = [
            sb.tile([128, SQ], mmdt, tag=f"attnN{pr}", name=f"attnN{pr}")
            for pr in range(4)
        ]
        v = [
            sb.tile([128, H, DH + 1], mmdt, tag=f"v{t}", name=f"v{t}")
            for t in range(nsk)
        ]
        wu = sb.tile([128, 512], mmdt, tag="wu", name="wu")

        bvb = blv[:, 2048 + KW : 2048 + KW + D]
        ones8 = blv[:, 2048 + KW + D : 2048 + KW + D + 8]
        bob = blo[:, 2048 : 2048 + D]

        # ---- DMA emission, consumption order ----
        X = 2048  # x-region offset inside k/q/v blobs
        h_k = min(((s_pad // 2 + 127) // 128) * 128, s_pad)
        p_v0 = min(5 * 128, s_pad)  # xv piece 0 covers t=0..4

        def ld(dst, src):
            nc.sync.dma_start(dst, src)

        ld(bk[:], d_bk[:])
        ld(bq[:], d_bq[:])
        ld(mb[:], d_mb[:])
        for c0, cw in _chunks(2048, 1024):  # wk
            ld(blk[:, c0 : c0 + cw], d_bk_blob[:, c0 : c0 + cw])
        for c0, cw in _chunks(2048, 1024):  # wq
            ld(blq[:, c0 : c0 + cw], d_bq_blob[:, c0 : c0 + cw])
        for dk in range(4):  # xk first halves
            o = X + dk * s_pad
            ld(blk[:, o : o + h_k], d_bk_blob[:, o : o + h_k])
        for dk in range(4):  # xq first halves
            o = X + dk * SQ
            ld(blq[:, o : o + 512], d_bq_blob[:, o : o + 512])
        for c0, cw in _chunks(2048, 1024):  # wv
            ld(blv[:, c0 : c0 + cw], d_bv_blob[:, c0 : c0 + cw])
        ld(blv[:, X + KW :], d_bv_blob[:, X + KW :])  # v bias + ones
        for dk in range(4):  # xv piece 0 (t=0..4)
            o = X + dk * s_pad
            ld(blv[:, o : o + p_v0], d_bv_blob[:, o : o + p_v0])
        if h_k < s_pad:  # xk second halves
            for dk in range(4):
                o = X + dk * s_pad + h_k
                e = X + dk * s_pad + s_pad
                ld(blk[:, o:e], d_bk_blob[:, o:e])
        if p_v0 < s_pad:  # xv piece 1
            for dk in range(4):
                o = X + dk * s_pad + p_v0
                e = X + dk * s_pad + s_pad
                ld(blv[:, o:e], d_bv_blob[:, o:e])
        for dk in range(4):  # xq second halves
            o = X + dk * SQ + 512
            ld(blq[:, o : o + 512], d_bq_blob[:, o : o + 512])
        for c0, cw in _chunks(2048 + D, 1024):  # wo + output bias (~55us in)
            ld(blo[:, c0 : c0 + cw], d_bo_blob[:, c0 : c0 + cw])

        # ---- borrowed-psum allocator for injected units ----
        bctx = {"set": 1, "tog": 0, "n": 0}

        def unit_ps(shape=None):
            tag = f"av{bctx['set']}{'AB'[bctx['tog']]}"
            bctx["tog"] ^= 1
            bctx["n"] += 1
            return ps_av.tile(
                shape or [128, 512], f32, tag=tag, name=f"ups{bctx['n']}"
            )

        # ---- PE warm-up (no data deps; runs during preamble/DMA) ----
        nc.vector.memset(wu[:], 0.0)
        for i in range(14):
            wps = unit_ps()
            nc.tensor.matmul(wps[:], lhsT=wu[:, 0:128], rhs=wu[:], start=True, stop=True)

        # ---- projection emitters ----
        def kq_chunk(which, j, c0, cw):
            blob, bias_t, dst = (blk, bk, kT) if which == "k" else (blq, bq, qT)
            sp = s_pad if which == "k" else SQ
            ps = unit_ps()
            for dk in range(4):
                nc.tensor.matmul(
                    ps[:, 0:cw],
                    lhsT=blob[:, dk * 512 + j * 128 : dk * 512 + (j + 1) * 128],
                    rhs=blob[:, X + dk * sp + c0 : X + dk * sp + c0 + cw],
                    start=(dk == 0),
                    stop=(dk == 3),
                )
            nc.vector.tensor_scalar_add(
                dst[j][:, c0 : c0 + cw], ps[:, 0:cw], bias_t[:, j : j + 1]
            )

        def v_proj(t):
            ps = unit_ps()
            for dk in range(4):
                nc.tensor.matmul(
                    ps[:],
                    lhsT=blv[:, X + dk * s_pad + t * 128 : X + dk * s_pad + (t + 1) * 128],
                    rhs=blv[:, dk * 512 : (dk + 1) * 512],
                    start=(dk == 0),
                    stop=(dk == 3),
                )
            nc.vector.tensor_copy(
                v[t][:, :, DH : DH + 1], ones8.rearrange("p (h o) -> p h o", o=1)
            )
            nc.vector.scalar_tensor_tensor(
                v[t][:, :, 0:DH],
                ps[:].rearrange("p (h d) -> p h d", h=H),
                1.0,
                bvb.rearrange("p (h d) -> p h d", h=H),
                op0=mybir.AluOpType.mult,
                op1=mybir.AluOpType.add,
            )

        def o_drain(qc, qt, ops):
            qq = qc * 512 + qt * 128
            osb = osb_p.tile([128, D], f32, tag="osb", name=f"osb{qc}_{qt}")
            nc.vector.scalar_tensor_tensor(
                osb[:],
                ops[:],
                1.0,
                bob,
                op0=mybir.AluOpType.mult,
                op1=mybir.AluOpType.add,
            )
            # two half-DMAs so consecutive outputs land on different queues
            nc.sync.dma_start(d_out[qq : qq + 64, :], osb[0:64, :])
            nc.sync.dma_start(d_out[qq + 64 : qq + 128, :], osb[64:128, :])

        def o_unit(qc, qt, ps=None, prs=range(4), stop=True):
            qq = qc * 512 + qt * 128
            if ps is None:
                ps = unit_ps()
            for pr2 in prs:
                nc.tensor.matmul(
                    ps[:],
                    lhsT=attnN[pr2][:, qq : qq + 128],
                    rhs=blo[:, pr2 * 512 : (pr2 + 1) * 512],
                    start=(pr2 == 0),
                    stop=(pr2 == 3 and stop),
                )
            if stop:
                o_drain(qc, qt, ps)
            return ps

        # ---- unit schedule over the flat step sequence ----
        # Steps: 8 windows x nsk key blocks, one exp tile per step. AV pairs
        # lag two steps behind their exp (decouples av from the lead-in DMA
        # and carries the pipeline across window boundaries). Injected units
        # get (earliest, deadline) step constraints and are placed EDF.
        kch = _chunks(s_pad, 512)
        windows = [(qc, pr) for qc in range(2) for pr in range(4)]
        NW = len(windows)
        nstep = NW * nsk

        units = []  # (unit, earliest_step, deadline_step)
        E_XKH1 = 3 if h_k < s_pad else 0  # xk 2nd half lands ~step 3
        E_XVP1 = 4 if p_v0 < s_pad else 0
        E_XQH1 = 8  # xq 2nd half lands ~step 8
        for t in range(nsk):
            e = 0 if t < 5 else E_XVP1
            units.append((("v", t), e, max(t, e)))  # used by av at step t+2
        for j in range(4):
            for ci, (c0, cw) in enumerate(kch):
                if j == 0 and ci == 0:
                    continue  # up-front
                use = 0 * nsk + c0 // 128 if j == 0 else j * nsk + c0 // 128
                e = E_XKH1 if c0 >= h_k else 0
                units.append((("k", j, c0, cw), e, max(use - 2, e)))
            if j > 0:
                units.append((("q", j, 0, 512), 0, j * nsk - 2))
            units.append((("q", j, 512, 512), E_XQH1, max((4 + j) * nsk - 2, E_XQH1)))

        pinned = {}  # step -> list of units
        for qt in range(4):  # O(0) inside window (1,0), after finish(0,3)
            pinned.setdefault(4 * nsk + 3 + qt, []).append(("o", 0, qt))
        for qt in (0, 1):  # O(1) qt0/qt1 partial pre-accumulation in (1,3)
            pinned.setdefault(7 * nsk + 3 + qt, []).append(("o1p", 1, qt))

        # usable slots: window 0 all steps; later windows steps >= 3 (the
        # borrowed av set is still being read by the previous window's
        # normalization during steps 0-2)
        free = []
        for wi in range(NW):
            s0 = 0 if wi == 0 else 3
            for t in range(s0, nsk):
                st = wi * nsk + t
                if st not in pinned:
                    free.append(st)
        cap = {st: (2 if st >= 3 else 1) if st < nsk else 1 for st in free}
        load = {st: 0 for st in free}
        plan = {st: [] for st in range(nstep)}
        for st, us in pinned.items():
            plan[st] = list(us)
        for u, e, dl in sorted(units, key=lambda x: x[2]):
            cands = [s for s in free if e <= s <= dl and load[s] < cap[s]]
            if cands:
                s = max(cands)  # latest-fit: don't front-load early windows
            else:
                cands = [s for s in free if e <= s <= dl]
                s = max(cands) if cands else min(s for s in free if s >= e)
            plan[s].append(u)
            load[s] += 1

        o1_ps = {}

        def run_unit(u):
            if u[0] == "v":
                v_proj(u[1])
            elif u[0] in ("k", "q"):
                kq_chunk(u[0], u[1], u[2], u[3])
            elif u[0] == "o":
                o_unit(u[1], u[2])
            elif u[0] == "o1p":  # O(1) partial pre-accumulation (pr 0..2)
                o1_ps[u[2]] = o_unit(u[1], u[2], prs=range(3), stop=False)

        # ---- up-front projections ----
        kq_chunk("k", 0, 0, 512)
        kq_chunk("q", 0, 0, 512)

        # ---- continuous attention pipeline ----
        AVLAG = 2
        wctx = {}  # wi -> (avA, avB, hA, hB, q0)

        def emit_av(wi, t, wx):
            avA, avB, hA, hB, q0 = wctx[wi]
            last = t == nsk - 1
            nc.tensor.matmul(
                avA[0:65, :],
                lhsT=v[t][:, hA : hA + 1, 0 : DH + 1],
                rhs=wx[:, 0:512],
                start=(t == 0),
                stop=last,
            )
            nc.tensor.matmul(
                avB[0:65, :],
                lhsT=v[t][:, hB : hB + 1, 0 : DH + 1],
                rhs=wx[:, 512:1024],
                start=(t == 0),
                stop=last,
            )
            if last:
                finish(wi)

        def finish(wi):
            avA, avB, hA, hB, q0 = wctx.pop(wi)
            qc, pr = windows[wi]
            # normalization (reciprocal needs SBUF input: copy denom rows out)
            dsA = r_p.tile([1, 512], f32, tag="dsA", name=f"dsA{qc}{pr}")
            dsB = r_p.tile([1, 512], f32, tag="dsB", name=f"dsB{qc}{pr}")
            nc.vector.tensor_copy(dsA[0:1, :], avA[64:65, :])
            nc.vector.tensor_copy(dsB[0:1, :], avB[64:65, :])
            rfA = r_p.tile([1, 512], f32, tag="rfA", name=f"rfA{qc}{pr}")
            rfB = r_p.tile([1, 512], f32, tag="rfB", name=f"rfB{qc}{pr}")
            nc.vector.reciprocal_approx_fast(rfA[0:1, :], dsA[0:1, :])
            nc.vector.reciprocal_approx_fast(rfB[0:1, :], dsB[0:1, :])
            bcsA = r_p.tile([64, 512], f32, tag="bcsA", name=f"bcsA{qc}{pr}")
            bcsB = r_p.tile([64, 512], f32, tag="bcsB", name=f"bcsB{qc}{pr}")
            nc.gpsimd.partition_broadcast(bcsA[0:64, :], rfA[0:1, :], channels=64)
            nc.gpsimd.partition_broadcast(bcsB[0:64, :], rfB[0:1, :], channels=64)
            nc.vector.tensor_mul(
                attnN[pr][0:64, q0 : q0 + 512], avA[0:64, :], bcsA[0:64, :]
            )
            nc.vector.tensor_mul(
                attnN[pr][64:128, q0 : q0 + 512], avB[0:64, :], bcsB[0:64, :]
            )

        hist = []  # emitted (wi, t, wx) awaiting their av pair
        for step in range(nstep):
            wi, t = divmod(step, nsk)
            qc, pr = windows[wi]
            if t == 0:
                st = wi % 2
                wctx[wi] = (
                    ps_av.tile([65, 512], f32, tag=f"av{st}A", name=f"avA{qc}{pr}"),
                    ps_av.tile([65, 512], f32, tag=f"av{st}B", name=f"avB{qc}{pr}"),
                    2 * pr,
                    2 * pr + 1,
                    qc * 512,
                )
            bctx["set"] = 1 - (wi % 2)
            lg = ps_lg.tile([128, 1024], f32, tag="lg", name=f"lg{qc}{pr}_{t}")
            q0 = qc * 512
            nc.tensor.matmul(
                lg[:, 0:512],
                lhsT=kT[pr][0:64, t * 128 : (t + 1) * 128],
                rhs=qT[pr][0:64, q0 : q0 + 512],
                start=True,
                stop=True,
            )
            nc.tensor.matmul(
                lg[:, 512:1024],
                lhsT=kT[pr][64:128, t * 128 : (t + 1) * 128],
                rhs=qT[pr][64:128, q0 : q0 + 512],
                start=True,
                stop=True,
            )
            wx = wexp_p.tile([128, 1024], mmdt, tag="wexp", name=f"wx{qc}{pr}_{t}")
            nc.scalar.activation(wx[:], lg[:], EXP, bias=mb[:, t : t + 1], scale=SCALE)
            hist.append((wi, t, wx))
            if len(hist) > AVLAG:
                emit_av(*hist.pop(0))
            for u in plan[step]:
                run_unit(u)
        for h in hist:
            emit_av(*h)

        # ---- tail: finalize O(1) ----
        for qt in (0, 1):  # pre-accumulated: one matmul (pr=3) + drain
            ps = o1_ps[qt]
            qq = 512 + qt * 128
            nc.tensor.matmul(
                ps[:],
                lhsT=attnN[3][:, qq : qq + 128],
                rhs=blo[:, 3 * 512 : 4 * 512],
                start=False,
                stop=True,
            )
            o_drain(1, qt, ps)
        for qt in (2, 3):  # full O units on now-free lg slots
            ps = ps_lg.tile([128, 1024], f32, tag="lg", name=f"o1f{qt}")[:, 0:512]
            o_unit(1, qt, ps=ps)

    nc.compile()
    return nc


def _prep_inputs(query, key, value, mask, wq_w, wq_b, wk_w, wk_b, wv_w, wv_b, wo_w, wo_b,
                 mm_dtype="bf16"):
    import ml_dtypes

    od = {"bf16": ml_dtypes.bfloat16, "f32r": np.float32, "f32": np.float32}[mm_dtype]
    f = lambda a: np.ascontiguousarray(np.asarray(a, dtype=np.float32))
    g = lambda a: np.ascontiguousarray(np.asarray(a).astype(od))
    query, key, value = f(query), f(key), f(value)
    wq_w, wk_w, wv_w, wo_w = f(wq_w), f(wk_w), f(wv_w), f(wo_w)
    mask = np.asarray(mask)

    keeps = [np.flatnonzero(mask[b] == 0) for b in range(B)]
    cnts = [len(k) for k in keeps]
    assert min(cnts) > 0, "all-masked batch not supported"
    s_pad = max(128, ((max(cnts) + 127) // 128) * 128)
    nsk = s_pad // 128

    bq_pp = np.ascontiguousarray(f(wq_b).reshape(4, 128).T)
    bk_pp = np.ascontiguousarray(f(wk_b).reshape(4, 128).T)
    bvb = np.broadcast_to(f(wv_b).reshape(1, D), (128, D))
    bob = np.broadcast_to(f(wo_b).reshape(1, D), (128, D))

    def wchunks(w):
        # [512, 512] -> [128, 4*512]: col block dk holds rows dk*128..dk*128+128
        return w.reshape(4, 128, D).transpose(1, 0, 2).reshape(128, 4 * D)

    def xchunks(xt):
        # [512, S] -> [128, 4*S]
        s = xt.shape[1]
        return xt.reshape(4, 128, s).transpose(1, 0, 2).reshape(128, 4 * s)

    blob_o = np.concatenate([wchunks(f(wo_w)), bob], axis=1)

    common = dict(
        bq_pp=bq_pp, bk_pp=bk_pp,
        blob_o=g(blob_o),
    )
    in_maps = []
    for b in range(B):
        kc = np.zeros((s_pad, D), np.float32)
        kc[: cnts[b]] = key[b][keeps[b]]
        vc = np.zeros((s_pad, D), np.float32)
        vc[: cnts[b]] = value[b][keeps[b]]
        blob_k = g(np.concatenate([wchunks(f(wk_w)), xchunks(kc.T)], axis=1))
        blob_v = g(
            np.concatenate(
                [
                    wchunks(f(wv_w)),
                    xchunks(vc.T),
                    bvb,
                    np.ones((128, 8), np.float32),
                ],
                axis=1,
            )
        )
        mbf = np.zeros(s_pad, np.float32)
        mbf[cnts[b] :] = -1e9
        mbd = np.ascontiguousarray(mbf.reshape(nsk, 128).T)
        for qh in range(2):
            blob_q = g(
                np.concatenate(
                    [wchunks(f(wq_w)), xchunks(query[b, qh * SQ : (qh + 1) * SQ, :].T)],
                    axis=1,
                )
            )
            in_maps.append(
                dict(blob_k=blob_k, blob_q=blob_q, blob_v=blob_v, mb=mbd, **common)
            )
    return s_pad, in_maps


def kernel(**inputs):
    from concourse import bass_utils

    mmd = os.environ.get("BASSK_MMDT", "bf16")
    s_pad, in_maps = _prep_inputs(**inputs, mm_dtype=mmd)
    key = (s_pad, mmd)
    if key not in _BUILT:
        _BUILT[key] = build_bass(s_pad, mm_dtype=key[1])
    nc = _BUILT[key]
    kw = {}
    if os.environ.get("BASSK_TRACE"):
        kw = dict(trace=True, stitch_traces=False)
    res = bass_utils.run_bass_kernel_spmd(nc, in_maps, core_ids=list(range(NCORE)), **kw)
    out = np.empty((B, S, D), np.float32)
    for c in range(NCORE):
        b, qh = c // 2, c % 2
        out[b, qh * SQ : (qh + 1) * SQ, :] = res.results[c]["out"]
    kernel.last_result = res
    return out


# revision 19
# speedup vs baseline: 1.2735x; 1.0420x over previous
"""Multi-head attention (B=4, S=2048, D=512, H=8) on 8 trn2 NeuronCores.

Sharding: core c = (batch b = c//2, query-half qh = c%2). Each core computes
the full attention output for 1024 query rows of one batch element.

v2.1: single fully-pipelined phase. The attention inner loop is ACT-bound
(exp of [128,1024] logit tiles, 72 of them); projection and O-projection
matmuls are injected into the PE stream inside the windows, filling the PE's
exp-wait slack. Window w+1 uses the opposite PSUM av tag-set from window w,
so w's normalization chain (reciprocal -> partition broadcast -> multiply)
never blocks w+1's AV matmuls. Injected units borrow the av tag-set not used
by the current window (slots >= 3 only, after the previous window's release).
The last window pre-accumulates half the O-projection and normalizes via a
K=1 PE broadcast to shorten the tail. DMA pieces are issued in consumption
order; warm-up matmuls keep HAM un-throttled from ~6us.

Device-side scheme (unchanged):
  - host supplies transposed activations (X^T layouts); Q^T/K^T projections
    with per-partition bias in the drain; V in natural layout [128, H, DH+1]
    with a ones column (softmax denominator rides in the AV matmul, M=65).
  - logits transposed: lg^T[s_k, q], two heads per PE pass (row-packed K=64).
  - exp on ACT: w = exp(0.125*lg + mb); masked/padded keys get bias -1e9.
  - AV accumulated per head over key blocks; normalization via reciprocal +
    partition broadcast; O projection streamed to HBM per 128 rows.

Masked keys (mask==1) are compacted away on the host (exact), keys padded to
a multiple of 128 with -1e9 mask bias.
"""

import os
import numpy as np

B, S, D, H = 4, 2048, 512, 8
DH = D // H
NCORE = 8
SQ = S // 2  # queries per core
SCALE = 1.0 / float(np.sqrt(DH))

_BUILT = {}


def _chunks(total, step):
    out = []
    c0 = 0
    while c0 < total:
        out.append((c0, min(step, total - c0)))
        c0 += step
    return out


def build_bass(s_pad, mm_dtype="bf16"):
    import concourse.bass as bass  # noqa: F401
    import concourse.mybir as mybir
    import concourse.tile as tile
    from concourse import bacc
    from contextlib import ExitStack

    f32 = mybir.dt.float32
    mmdt = {
        "bf16": mybir.dt.bfloat16,
        "f32r": mybir.dt.float32r,
        "f32": mybir.dt.float32,
    }[mm_dtype]
    EXP = mybir.ActivationFunctionType.Exp

    nsk = s_pad // 128

    nc = bacc.Bacc(
        "TRN2",
        target_bir_lowering=False,
        debug=False,
        enable_asserts=False,
        num_devices=NCORE,
    )

    KW, QW = 4 * s_pad, 4 * SQ
    d_bk_blob = nc.dram_tensor("blob_k", [128, 2048 + KW], mmdt, kind="ExternalInput").ap()
    d_bq_blob = nc.dram_tensor("blob_q", [128, 2048 + QW], mmdt, kind="ExternalInput").ap()
    d_bv_blob = nc.dram_tensor("blob_v", [128, 2048 + KW + D + 8], mmdt, kind="ExternalInput").ap()
    d_bo_blob = nc.dram_tensor("blob_o", [128, 2048 + D], mmdt, kind="ExternalInput").ap()
    d_mb = nc.dram_tensor("mb", [128, nsk], f32, kind="ExternalInput").ap()
    d_bq = nc.dram_tensor("bq_pp", [128, 4], f32, kind="ExternalInput").ap()
    d_bk = nc.dram_tensor("bk_pp", [128, 4], f32, kind="ExternalInput").ap()
    d_out = nc.dram_tensor("out", [SQ, D], f32, kind="ExternalOutput").ap()

    with tile.TileContext(nc) as tc, ExitStack() as ctx, nc.allow_low_precision(
        "matmul operands held in bf16 (tolerance 2e-2; measured ~6e-3)"
    ):
        sb = ctx.enter_context(tc.tile_pool(name="sb", bufs=1))
        # PSUM budget (16KB/partition): lg 2x[128,1024]f32 = 8KB;
        # av sets 0/1 x (A,B) 1 buf x [128,512]f32 = 8KB. Injected units
        # borrow slots from the av set the current window is NOT using.
        ps_lg = ctx.enter_context(tc.tile_pool(name="pslg", bufs=2, space="PSUM"))
        ps_av = ctx.enter_context(tc.tile_pool(name="psav", bufs=1, space="PSUM"))
        wexp_p = ctx.enter_context(tc.tile_pool(name="wexp", bufs=4))
        osb_p = ctx.enter_context(tc.tile_pool(name="osb", bufs=2))
        r_p = ctx.enter_context(tc.tile_pool(name="rp", bufs=2))

        # ---- persistent SBUF tiles ----
        blk = sb.tile([128, 2048 + KW], mmdt, tag="blk", name="blk")
        blq = sb.tile([128, 2048 + QW], mmdt, tag="blq", name="blq")
        blv = sb.tile([128, 2048 + KW + D + 8], mmdt, tag="blv", name="blv")
        blo = sb.tile([128, 2048 + D], mmdt, tag="blo", name="blo")
        bk = sb.tile([128, 4], f32, tag="bk", name="bk")
        bq = sb.tile([128, 4], f32, tag="bq", name="bq")
        mb = sb.tile([128, nsk], f32, tag="mb", name="mb")
        kT = [sb.tile([128, s_pad], mmdt, tag=f"kT{j}", name=f"kT{j}") for j in range(4)]
        qT = [sb.tile([128, SQ], mmdt, tag=f"qT{j}", name=f"qT{j}") for j in range(4)]
        attnN = [
            sb.tile([128, SQ], mmdt, tag=f"attnN{pr}", name=f"attnN{pr}")
            for pr in range(4)
        ]
        v = [
            sb.tile([128, H, DH + 1], mmdt, tag=f"v{t}", name=f"v{t}")
            for t in range(nsk)
        ]
        wu = sb.tile([128, 512], mmdt, tag="wu", name="wu")

        bvb = blv[:, 2048 + KW : 2048 + KW + D]
        ones8 = blv[:, 2048 + KW + D : 2048 + KW + D + 8]
        bob = blo[:, 2048 : 2048 + D]

        # ---- DMA emission, consumption order ----
        X = 2048  # x-region offset inside k/q/v blobs
        h_k = min(((s_pad // 2 + 127) // 128) * 128, s_pad)
        p_v0 = min(5 * 128, s_pad)  # xv piece 0 covers t=0..4

        def ld(dst, src):
            nc.sync.dma_start(dst, src)

        ld(bk[:], d_bk[:])
        ld(bq[:], d_bq[:])
        ld(mb[:], d_mb[:])
        for c0, cw in _chunks(2048, 1024):  # wk
            ld(blk[:, c0 : c0 + cw], d_bk_blob[:, c0 : c0 + cw])
        for c0, cw in _chunks(2048, 1024):  # wq
            ld(blq[:, c0 : c0 + cw], d_bq_blob[:, c0 : c0 + cw])
        for dk in range(4):  # xk first halves
            o = X + dk * s_pad
            ld(blk[:, o : o + h_k], d_bk_blob[:, o : o + h_k])
        for dk in range(4):  # xq first halves
            o = X + dk * SQ
            ld(blq[:, o : o + 512], d_bq_blob[:, o : o + 512])
        for c0, cw in _chunks(2048, 1024):  # wv
            ld(blv[:, c0 : c0 + cw], d_bv_blob[:, c0 : c0 + cw])
        ld(blv[:, X + KW :], d_bv_blob[:, X + KW :])  # v bias + ones
        for dk in range(4):  # xv piece 0 (t=0..4)
            o = X + dk * s_pad
            ld(blv[:, o : o + p_v0], d_bv_blob[:, o : o + p_v0])
        if h_k < s_pad:  # xk second halves
            for dk in range(4):
                o = X + dk * s_pad + h_k
                e = X + dk * s_pad + s_pad
                ld(blk[:, o:e], d_bk_blob[:, o:e])
        if p_v0 < s_pad:  # xv piece 1
            for dk in range(4):
                o = X + dk * s_pad + p_v0
                e = X + dk * s_pad + s_pad
                ld(blv[:, o:e], d_bv_blob[:, o:e])
        for dk in range(4):  # xq second halves
            o = X + dk * SQ + 512
            ld(blq[:, o : o + 512], d_bq_blob[:, o : o + 512])
        for c0, cw in _chunks(2048 + D, 1024):  # wo + output bias (~55us in)
            ld(blo[:, c0 : c0 + cw], d_bo_blob[:, c0 : c0 + cw])

        # ---- borrowed-psum allocator for injected units ----
        bctx = {"set": 1, "tog": 0, "n": 0}

        def unit_ps(shape=None):
            tag = f"av{bctx['set']}{'AB'[bctx['tog']]}"
            bctx["tog"] ^= 1
            bctx["n"] += 1
            return ps_av.tile(
                shape or [128, 512], f32, tag=tag, name=f"ups{bctx['n']}"
            )

        # ---- PE warm-up (no data deps; runs during preamble/DMA) ----
        nc.vector.memset(wu[:], 0.0)
        for i in range(14):
            wps = unit_ps()
            nc.tensor.matmul(wps[:], lhsT=wu[:, 0:128], rhs=wu[:], start=True, stop=True)

        # ---- projection emitters ----
        def kq_chunk(which, j, c0, cw):
            blob, bias_t, dst = (blk, bk, kT) if which == "k" else (blq, bq, qT)
            sp = s_pad if which == "k" else SQ
            ps = unit_ps()
            for dk in range(4):
                nc.tensor.matmul(
                    ps[:, 0:cw],
                    lhsT=blob[:, dk * 512 + j * 128 : dk * 512 + (j + 1) * 128],
                    rhs=blob[:, X + dk * sp + c0 : X + dk * sp + c0 + cw],
                    start=(dk == 0),
                    stop=(dk == 3),
                )
            nc.vector.tensor_scalar_add(
                dst[j][:, c0 : c0 + cw], ps[:, 0:cw], bias_t[:, j : j + 1]
            )

        def v_proj(t):
            ps = unit_ps()
            for dk in range(4):
                nc.tensor.matmul(
                    ps[:],
                    lhsT=blv[:, X + dk * s_pad + t * 128 : X + dk * s_pad + (t + 1) * 128],
                    rhs=blv[:, dk * 512 : (dk + 1) * 512],
                    start=(dk == 0),
                    stop=(dk == 3),
                )
            nc.vector.tensor_copy(
                v[t][:, :, DH : DH + 1], ones8.rearrange("p (h o) -> p h o", o=1)
            )
            nc.vector.scalar_tensor_tensor(
                v[t][:, :, 0:DH],
                ps[:].rearrange("p (h d) -> p h d", h=H),
                1.0,
                bvb.rearrange("p (h d) -> p h d", h=H),
                op0=mybir.AluOpType.mult,
                op1=mybir.AluOpType.add,
            )

        def o_drain(qc, qt, ops):
            qq = qc * 512 + qt * 128
            osb = osb_p.tile([128, D], f32, tag="osb", name=f"osb{qc}_{qt}")
            nc.vector.scalar_tensor_tensor(
                osb[:],
                ops[:],
                1.0,
                bob,
                op0=mybir.AluOpType.mult,
                op1=mybir.AluOpType.add,
            )
            nc.sync.dma_start(d_out[qq : qq + 128, :], osb[:])

        def o_unit(qc, qt, ps=None, prs=range(4), stop=True):
            qq = qc * 512 + qt * 128
            if ps is None:
                ps = unit_ps()
            for pr2 in prs:
                nc.tensor.matmul(
                    ps[:],
                    lhsT=attnN[pr2][:, qq : qq + 128],
                    rhs=blo[:, pr2 * 512 : (pr2 + 1) * 512],
                    start=(pr2 == 0),
                    stop=(pr2 == 3 and stop),
                )
            if stop:
                o_drain(qc, qt, ps)
            return ps

        # ---- unit schedule over the flat step sequence ----
        # Steps: 8 windows x nsk key blocks, one exp tile per step. AV pairs
        # lag two steps behind their exp (decouples av from the lead-in DMA
        # and carries the pipeline across window boundaries). Injected units
        # get (earliest, deadline) step constraints and are placed EDF.
        kch = _chunks(s_pad, 512)
        windows = [(qc, pr) for qc in range(2) for pr in range(4)]
        NW = len(windows)
        nstep = NW * nsk

        units = []  # (unit, earliest_step, deadline_step)
        E_XKH1 = 3 if h_k < s_pad else 0  # xk 2nd half lands ~step 3
        E_XVP1 = 4 if p_v0 < s_pad else 0
        E_XQH1 = 8  # xq 2nd half lands ~step 8
        for t in range(nsk):
            e = 0 if t < 5 else E_XVP1
            units.append((("v", t), e, max(t, e)))  # used by av at step t+2
        for j in range(4):
            for ci, (c0, cw) in enumerate(kch):
                if j == 0 and ci == 0:
                    continue  # up-front
                use = 0 * nsk + c0 // 128 if j == 0 else j * nsk + c0 // 128
                e = E_XKH1 if c0 >= h_k else 0
                units.append((("k", j, c0, cw), e, max(use - 2, e)))
            if j > 0:
                units.append((("q", j, 0, 512), 0, j * nsk - 2))
            units.append((("q", j, 512, 512), E_XQH1, max((4 + j) * nsk - 2, E_XQH1)))

        pinned = {}  # step -> list of units
        for qt in range(4):  # O(0) inside window (1,0), after finish(0,3)
            pinned.setdefault(4 * nsk + min(5 + qt, nsk - 1), []).append(("o", 0, qt))
        for qt in (0, 1):  # O(1) qt0/qt1 partial pre-accumulation in (1,3)
            pinned.setdefault(7 * nsk + min(5 + qt, nsk - 1), []).append(("o1p", 1, qt))

        # usable slots: window 0 all steps; later windows steps >= 4 (the
        # borrowed av set is released by the previous window's normalization
        # chain only ~3 steps in)
        free = []
        for wi in range(NW):
            s0 = 0 if wi == 0 else 4
            for t in range(s0, nsk):
                st = wi * nsk + t
                if st not in pinned:
                    free.append(st)
        cap = {st: (2 if st >= 3 else 1) if st < nsk else 1 for st in free}
        load = {st: 0 for st in free}
        plan = {st: [] for st in range(nstep)}
        for st, us in pinned.items():
            plan[st] = list(us)
        for u, e, dl in sorted(units, key=lambda x: x[2]):
            cands = [s for s in free if e <= s <= dl and load[s] < cap[s]]
            if cands:
                s = max(cands)  # latest-fit: don't front-load early windows
            else:
                cands = [s for s in free if e <= s <= dl]
                s = max(cands) if cands else min(s for s in free if s >= e)
            plan[s].append(u)
            load[s] += 1

        o1_ps = {}

        def run_unit(u):
            if u[0] == "v":
                v_proj(u[1])
            elif u[0] in ("k", "q"):
                kq_chunk(u[0], u[1], u[2], u[3])
            elif u[0] == "o":
                o_unit(u[1], u[2])
            elif u[0] == "o1p":  # O(1) partial pre-accumulation (pr 0..2)
                o1_ps[u[2]] = o_unit(u[1], u[2], prs=range(3), stop=False)

        # ---- up-front projections ----
        kq_chunk("k", 0, 0, 512)
        kq_chunk("q", 0, 0, 512)

        # ---- continuous attention pipeline ----
        AVLAG = 2
        wctx = {}  # wi -> (avA, avB, hA, hB, q0)

        def emit_av(wi, t, wx):
            avA, avB, hA, hB, q0 = wctx[wi]
            last = t == nsk - 1
            nc.tensor.matmul(
                avA[0:65, :],
                lhsT=v[t][:, hA : hA + 1, 0 : DH + 1],
                rhs=wx[:, 0:512],
                start=(t == 0),
                stop=last,
            )
            nc.tensor.matmul(
                avB[0:65, :],
                lhsT=v[t][:, hB : hB + 1, 0 : DH + 1],
                rhs=wx[:, 512:1024],
                start=(t == 0),
                stop=last,
            )
            if last:
                finish(wi)

        def finish(wi):
            avA, avB, hA, hB, q0 = wctx.pop(wi)
            qc, pr = windows[wi]
            # normalization (reciprocal needs SBUF input: copy denom rows out)
            dsA = r_p.tile([1, 512], f32, tag="dsA", name=f"dsA{qc}{pr}")
            dsB = r_p.tile([1, 512], f32, tag="dsB", name=f"dsB{qc}{pr}")
            nc.vector.tensor_copy(dsA[0:1, :], avA[64:65, :])
            nc.vector.tensor_copy(dsB[0:1, :], avB[64:65, :])
            rfA = r_p.tile([1, 512], f32, tag="rfA", name=f"rfA{qc}{pr}")
            rfB = r_p.tile([1, 512], f32, tag="rfB", name=f"rfB{qc}{pr}")
            nc.vector.reciprocal_approx_fast(rfA[0:1, :], dsA[0:1, :])
            nc.vector.reciprocal_approx_fast(rfB[0:1, :], dsB[0:1, :])
            bcsA = r_p.tile([64, 512], f32, tag="bcsA", name=f"bcsA{qc}{pr}")
            bcsB = r_p.tile([64, 512], f32, tag="bcsB", name=f"bcsB{qc}{pr}")
            nc.gpsimd.partition_broadcast(bcsA[0:64, :], rfA[0:1, :], channels=64)
            nc.gpsimd.partition_broadcast(bcsB[0:64, :], rfB[0:1, :], channels=64)
            nc.vector.tensor_mul(
                attnN[pr][0:64, q0 : q0 + 512], avA[0:64, :], bcsA[0:64, :]
            )
            nc.vector.tensor_mul(
                attnN[pr][64:128, q0 : q0 + 512], avB[0:64, :], bcsB[0:64, :]
            )

        hist = []  # emitted (wi, t, wx) awaiting their av pair
        for step in range(nstep):
            wi, t = divmod(step, nsk)
            qc, pr = windows[wi]
            if t == 0:
                st = wi % 2
                wctx[wi] = (
                    ps_av.tile([65, 512], f32, tag=f"av{st}A", name=f"avA{qc}{pr}"),
                    ps_av.tile([65, 512], f32, tag=f"av{st}B", name=f"avB{qc}{pr}"),
                    2 * pr,
                    2 * pr + 1,
                    qc * 512,
                )
            bctx["set"] = 1 - (wi % 2)
            lg = ps_lg.tile([128, 1024], f32, tag="lg", name=f"lg{qc}{pr}_{t}")
            q0 = qc * 512
            nc.tensor.matmul(
                lg[:, 0:512],
                lhsT=kT[pr][0:64, t * 128 : (t + 1) * 128],
                rhs=qT[pr][0:64, q0 : q0 + 512],
                start=True,
                stop=True,
            )
            nc.tensor.matmul(
                lg[:, 512:1024],
                lhsT=kT[pr][64:128, t * 128 : (t + 1) * 128],
                rhs=qT[pr][64:128, q0 : q0 + 512],
                start=True,
                stop=True,
            )
            wx = wexp_p.tile([128, 1024], mmdt, tag="wexp", name=f"wx{qc}{pr}_{t}")
            nc.scalar.activation(wx[:], lg[:], EXP, bias=mb[:, t : t + 1], scale=SCALE)
            hist.append((wi, t, wx))
            if len(hist) > AVLAG:
                emit_av(*hist.pop(0))
            for u in plan[step]:
                run_unit(u)
        # qt2 partial on the first now-free lg slot (runs while the last
        # exps drain), then flush the remaining av pairs + final finish
        o1_ps[2] = o_unit(
            1, 2,
            ps=ps_lg.tile([128, 1024], f32, tag="lg", name="o1f2")[:, 0:512],
            prs=range(3), stop=False,
        )
        for h in hist:
            emit_av(*h)
        o1_ps[3] = o_unit(
            1, 3,
            ps=ps_lg.tile([128, 1024], f32, tag="lg", name="o1f3")[:, 0:512],
            prs=range(3), stop=False,
        )

        # ---- tail: finalize O(1): one matmul (pr=3) + drain per qt ----
        for qt in range(4):
            ps = o1_ps[qt]
            qq = 512 + qt * 128
            nc.tensor.matmul(
                ps[:],
                lhsT=attnN[3][:, qq : qq + 128],
                rhs=blo[:, 3 * 512 : 4 * 512],
                start=False,
                stop=True,
            )
            o_drain(1, qt, ps)

    nc.compile()
    return nc


def _prep_inputs(query, key, value, mask, wq_w, wq_b, wk_w, wk_b, wv_w, wv_b, wo_w, wo_b,
                 mm_dtype="bf16"):
    import ml_dtypes

    od = {"bf16": ml_dtypes.bfloat16, "f32r": np.float32, "f32": np.float32}[mm_dtype]
    f = lambda a: np.ascontiguousarray(np.asarray(a, dtype=np.float32))
    g = lambda a: np.ascontiguousarray(np.asarray(a).astype(od))
    query, key, value = f(query), f(key), f(value)
    wq_w, wk_w, wv_w, wo_w = f(wq_w), f(wk_w), f(wv_w), f(wo_w)
    mask = np.asarray(mask)

    keeps = [np.flatnonzero(mask[b] == 0) for b in range(B)]
    cnts = [len(k) for k in keeps]
    assert min(cnts) > 0, "all-masked batch not supported"
    s_pad = max(128, ((max(cnts) + 127) // 128) * 128)
    nsk = s_pad // 128

    bq_pp = np.ascontiguousarray(f(wq_b).reshape(4, 128).T)
    bk_pp = np.ascontiguousarray(f(wk_b).reshape(4, 128).T)
    bvb = np.broadcast_to(f(wv_b).reshape(1, D), (128, D))
    bob = np.broadcast_to(f(wo_b).reshape(1, D), (128, D))

    def wchunks(w):
        # [512, 512] -> [128, 4*512]: col block dk holds rows dk*128..dk*128+128
        return w.reshape(4, 128, D).transpose(1, 0, 2).reshape(128, 4 * D)

    def xchunks(xt):
        # [512, S] -> [128, 4*S]
        s = xt.shape[1]
        return xt.reshape(4, 128, s).transpose(1, 0, 2).reshape(128, 4 * s)

    blob_o = np.concatenate([wchunks(f(wo_w)), bob], axis=1)

    common = dict(
        bq_pp=bq_pp, bk_pp=bk_pp,
        blob_o=g(blob_o),
    )
    in_maps = []
    for b in range(B):
        kc = np.zeros((s_pad, D), np.float32)
        kc[: cnts[b]] = key[b][keeps[b]]
        vc = np.zeros((s_pad, D), np.float32)
        vc[: cnts[b]] = value[b][keeps[b]]
        blob_k = g(np.concatenate([wchunks(f(wk_w)), xchunks(kc.T)], axis=1))
        blob_v = g(
            np.concatenate(
                [
                    wchunks(f(wv_w)),
                    xchunks(vc.T),
                    bvb,
                    np.ones((128, 8), np.float32),
                ],
                axis=1,
            )
        )
        mbf = np.zeros(s_pad, np.float32)
        mbf[cnts[b] :] = -1e9
        mbd = np.ascontiguousarray(mbf.reshape(nsk, 128).T)
        for qh in range(2):
            blob_q = g(
                np.concatenate(
                    [wchunks(f(wq_w)), xchunks(query[b, qh * SQ : (qh + 1) * SQ, :].T)],
                    axis=1,
                )
            )
            in_maps.append(
                dict(blob_k=blob_k, blob_q=blob_q, blob_v=blob_v, mb=mbd, **common)
            )
    return s_pad, in_maps


def kernel(**inputs):
    from concourse import bass_utils

    mmd = os.environ.get("BASSK_MMDT", "bf16")
    s_pad, in_maps = _prep_inputs(**inputs, mm_dtype=mmd)
    key = (s_pad, mmd)
    if key not in _BUILT:
        _BUILT[key] = build_bass(s_pad, mm_dtype=key[1])
    nc = _BUILT[key]
    kw = {}
    if os.environ.get("BASSK_TRACE"):
        kw = dict(trace=True, stitch_traces=False)
    res = bass_utils.run_bass_kernel_spmd(nc, in_maps, core_ids=list(range(NCORE)), **kw)
    out = np.empty((B, S, D), np.float32)
    for c in range(NCORE):
        b, qh = c // 2, c % 2
        out[b, qh * SQ : (qh + 1) * SQ, :] = res.results[c]["out"]
    kernel.last_result = res
    return out


# revision 20
# speedup vs baseline: 1.2824x; 1.0070x over previous
"""Multi-head attention (B=4, S=2048, D=512, H=8) on 8 trn2 NeuronCores.

Sharding: core c = (batch b = c//2, query-half qh = c%2). Each core computes
the full attention output for 1024 query rows of one batch element.

v2.1: single fully-pipelined phase. The attention inner loop is ACT-bound
(exp of [128,1024] logit tiles, 72 of them); projection and O-projection
matmuls are injected into the PE stream inside the windows, filling the PE's
exp-wait slack. Window w+1 uses the opposite PSUM av tag-set from window w,
so w's normalization chain (reciprocal -> partition broadcast -> multiply)
never blocks w+1's AV matmuls. Injected units borrow the av tag-set not used
by the current window (slots >= 3 only, after the previous window's release).
The last window pre-accumulates half the O-projection and normalizes via a
K=1 PE broadcast to shorten the tail. DMA pieces are issued in consumption
order; warm-up matmuls keep HAM un-throttled from ~6us.

Device-side scheme (unchanged):
  - host supplies transposed activations (X^T layouts); Q^T/K^T projections
    with per-partition bias in the drain; V in natural layout [128, H, DH+1]
    with a ones column (softmax denominator rides in the AV matmul, M=65).
  - logits transposed: lg^T[s_k, q], two heads per PE pass (row-packed K=64).
  - exp on ACT: w = exp(0.125*lg + mb); masked/padded keys get bias -1e9.
  - AV accumulated per head over key blocks; normalization via reciprocal +
    partition broadcast; O projection streamed to HBM per 128 rows.

Masked keys (mask==1) are compacted away on the host (exact), keys padded to
a multiple of 128 with -1e9 mask bias.
"""

import os
import numpy as np

B, S, D, H = 4, 2048, 512, 8
DH = D // H
NCORE = 8
SQ = S // 2  # queries per core
SCALE = 1.0 / float(np.sqrt(DH))

_BUILT = {}


def _chunks(total, step):
    out = []
    c0 = 0
    while c0 < total:
        out.append((c0, min(step, total - c0)))
        c0 += step
    return out


def build_bass(s_pad, mm_dtype="bf16"):
    import concourse.bass as bass  # noqa: F401
    import concourse.mybir as mybir
    import concourse.tile as tile
    from concourse import bacc
    from contextlib import ExitStack

    f32 = mybir.dt.float32
    mmdt = {
        "bf16": mybir.dt.bfloat16,
        "f32r": mybir.dt.float32r,
        "f32": mybir.dt.float32,
    }[mm_dtype]
    EXP = mybir.ActivationFunctionType.Exp

    nsk = s_pad // 128

    nc = bacc.Bacc(
        "TRN2",
        target_bir_lowering=False,
        debug=False,
        enable_asserts=False,
        num_devices=NCORE,
    )

    KW, QW = 4 * s_pad, 4 * SQ
    d_bk_blob = nc.dram_tensor("blob_k", [128, 2048 + KW], mmdt, kind="ExternalInput").ap()
    d_bq_blob = nc.dram_tensor("blob_q", [128, 2048 + QW], mmdt, kind="ExternalInput").ap()
    d_bv_blob = nc.dram_tensor("blob_v", [128, 2048 + KW + D + 8], mmdt, kind="ExternalInput").ap()
    d_bo_blob = nc.dram_tensor("blob_o", [128, 2048 + D], mmdt, kind="ExternalInput").ap()
    d_mb = nc.dram_tensor("mb", [128, nsk], f32, kind="ExternalInput").ap()
    d_bq = nc.dram_tensor("bq_pp", [128, 4], f32, kind="ExternalInput").ap()
    d_bk = nc.dram_tensor("bk_pp", [128, 4], f32, kind="ExternalInput").ap()
    d_out = nc.dram_tensor("out", [SQ, D], f32, kind="ExternalOutput").ap()

    with tile.TileContext(nc) as tc, ExitStack() as ctx, nc.allow_low_precision(
        "matmul operands held in bf16 (tolerance 2e-2; measured ~6e-3)"
    ):
        sb = ctx.enter_context(tc.tile_pool(name="sb", bufs=1))
        # PSUM budget (16KB/partition): lg 2x[128,1024]f32 = 8KB;
        # av sets 0/1 x (A,B) 1 buf x [128,512]f32 = 8KB. Injected units
        # borrow slots from the av set the current window is NOT using.
        ps_lg = ctx.enter_context(tc.tile_pool(name="pslg", bufs=2, space="PSUM"))
        ps_av = ctx.enter_context(tc.tile_pool(name="psav", bufs=1, space="PSUM"))
        wexp_p = ctx.enter_context(tc.tile_pool(name="wexp", bufs=4))
        osb_p = ctx.enter_context(tc.tile_pool(name="osb", bufs=4))
        r_p = ctx.enter_context(tc.tile_pool(name="rp", bufs=2))

        # ---- persistent SBUF tiles ----
        blk = sb.tile([128, 2048 + KW], mmdt, tag="blk", name="blk")
        blq = sb.tile([128, 2048 + QW], mmdt, tag="blq", name="blq")
        blv = sb.tile([128, 2048 + KW + D + 8], mmdt, tag="blv", name="blv")
        blo = sb.tile([128, 2048 + D], mmdt, tag="blo", name="blo")
        bk = sb.tile([128, 4], f32, tag="bk", name="bk")
        bq = sb.tile([128, 4], f32, tag="bq", name="bq")
        mb = sb.tile([128, nsk], f32, tag="mb", name="mb")
        kT = [sb.tile([128, s_pad], mmdt, tag=f"kT{j}", name=f"kT{j}") for j in range(4)]
        qT = [sb.tile([128, SQ], mmdt, tag=f"qT{j}", name=f"qT{j}") for j in range(4)]
        attnN = [
            sb.tile([128, SQ], mmdt, tag=f"attnN{pr}", name=f"attnN{pr}")
            for pr in range(4)
        ]
        v = [
            sb.tile([128, H, DH + 1], mmdt, tag=f"v{t}", name=f"v{t}")
            for t in range(nsk)
        ]
        wu = sb.tile([128, 512], mmdt, tag="wu", name="wu")

        bvb = blv[:, 2048 + KW : 2048 + KW + D]
        ones8 = blv[:, 2048 + KW + D : 2048 + KW + D + 8]
        bob = blo[:, 2048 : 2048 + D]

        # ---- DMA emission, consumption order ----
        X = 2048  # x-region offset inside k/q/v blobs
        h_k = min(1024, s_pad)
        p_v0 = min(5 * 128, s_pad)  # xv piece 0 covers t=0..4

        def ld(dst, src):
            nc.sync.dma_start(dst, src)

        ld(bk[:], d_bk[:])
        ld(bq[:], d_bq[:])
        ld(mb[:], d_mb[:])
        for c0, cw in _chunks(2048, 1024):  # wk
            ld(blk[:, c0 : c0 + cw], d_bk_blob[:, c0 : c0 + cw])
        for c0, cw in _chunks(2048, 1024):  # wq
            ld(blq[:, c0 : c0 + cw], d_bq_blob[:, c0 : c0 + cw])
        for dk in range(4):  # xk first halves
            o = X + dk * s_pad
            ld(blk[:, o : o + h_k], d_bk_blob[:, o : o + h_k])
        for dk in range(4):  # xq first halves
            o = X + dk * SQ
            ld(blq[:, o : o + 512], d_bq_blob[:, o : o + 512])
        for c0, cw in _chunks(2048, 1024):  # wv
            ld(blv[:, c0 : c0 + cw], d_bv_blob[:, c0 : c0 + cw])
        ld(blv[:, X + KW :], d_bv_blob[:, X + KW :])  # v bias + ones
        for dk in range(4):  # xv piece 0 (t=0..4)
            o = X + dk * s_pad
            ld(blv[:, o : o + p_v0], d_bv_blob[:, o : o + p_v0])
        if h_k < s_pad:  # xk second halves
            for dk in range(4):
                o = X + dk * s_pad + h_k
                e = X + dk * s_pad + s_pad
                ld(blk[:, o:e], d_bk_blob[:, o:e])
        if p_v0 < s_pad:  # xv piece 1
            for dk in range(4):
                o = X + dk * s_pad + p_v0
                e = X + dk * s_pad + s_pad
                ld(blv[:, o:e], d_bv_blob[:, o:e])
        for dk in range(4):  # xq second halves
            o = X + dk * SQ + 512
            ld(blq[:, o : o + 512], d_bq_blob[:, o : o + 512])
        for c0, cw in _chunks(2048 + D, 1024):  # wo + output bias (~55us in)
            ld(blo[:, c0 : c0 + cw], d_bo_blob[:, c0 : c0 + cw])

        # ---- borrowed-psum allocator for injected units ----
        bctx = {"set": 1, "tog": 0, "n": 0}

        def unit_ps(shape=None):
            tag = f"av{bctx['set']}{'AB'[bctx['tog']]}"
            bctx["tog"] ^= 1
            bctx["n"] += 1
            return ps_av.tile(
                shape or [128, 512], f32, tag=tag, name=f"ups{bctx['n']}"
            )

        # ---- PE warm-up (no data deps; runs during preamble/DMA) ----
        nc.vector.memset(wu[:], 0.0)
        for i in range(14):
            wps = unit_ps()
            nc.tensor.matmul(wps[:], lhsT=wu[:, 0:128], rhs=wu[:], start=True, stop=True)

        # ---- projection emitters ----
        def kq_chunk(which, j, c0, cw):
            blob, bias_t, dst = (blk, bk, kT) if which == "k" else (blq, bq, qT)
            sp = s_pad if which == "k" else SQ
            ps = unit_ps()
            for dk in range(4):
                nc.tensor.matmul(
                    ps[:, 0:cw],
                    lhsT=blob[:, dk * 512 + j * 128 : dk * 512 + (j + 1) * 128],
                    rhs=blob[:, X + dk * sp + c0 : X + dk * sp + c0 + cw],
                    start=(dk == 0),
                    stop=(dk == 3),
                )
            nc.vector.tensor_scalar_add(
                dst[j][:, c0 : c0 + cw], ps[:, 0:cw], bias_t[:, j : j + 1]
            )

        def v_proj(t):
            ps = unit_ps()
            for dk in range(4):
                nc.tensor.matmul(
                    ps[:],
                    lhsT=blv[:, X + dk * s_pad + t * 128 : X + dk * s_pad + (t + 1) * 128],
                    rhs=blv[:, dk * 512 : (dk + 1) * 512],
                    start=(dk == 0),
                    stop=(dk == 3),
                )
            nc.vector.tensor_copy(
                v[t][:, :, DH : DH + 1], ones8.rearrange("p (h o) -> p h o", o=1)
            )
            nc.vector.scalar_tensor_tensor(
                v[t][:, :, 0:DH],
                ps[:].rearrange("p (h d) -> p h d", h=H),
                1.0,
                bvb.rearrange("p (h d) -> p h d", h=H),
                op0=mybir.AluOpType.mult,
                op1=mybir.AluOpType.add,
            )

        def o_drain(qc, qt, ops):
            qq = qc * 512 + qt * 128
            osb = osb_p.tile([128, D], f32, tag="osb", name=f"osb{qc}_{qt}")
            nc.vector.scalar_tensor_tensor(
                osb[:],
                ops[:],
                1.0,
                bob,
                op0=mybir.AluOpType.mult,
                op1=mybir.AluOpType.add,
            )
            nc.sync.dma_start(d_out[qq : qq + 128, :], osb[:])

        def o_unit(qc, qt, ps=None, prs=range(4), stop=True):
            qq = qc * 512 + qt * 128
            if ps is None:
                ps = unit_ps()
            for pr2 in prs:
                nc.tensor.matmul(
                    ps[:],
                    lhsT=attnN[pr2][:, qq : qq + 128],
                    rhs=blo[:, pr2 * 512 : (pr2 + 1) * 512],
                    start=(pr2 == 0),
                    stop=(pr2 == 3 and stop),
                )
            if stop:
                o_drain(qc, qt, ps)
            return ps

        # ---- unit schedule over the flat step sequence ----
        # Steps: 8 windows x nsk key blocks, one exp tile per step. AV pairs
        # lag two steps behind their exp (decouples av from the lead-in DMA
        # and carries the pipeline across window boundaries). Injected units
        # get (earliest, deadline) step constraints and are placed EDF.
        kch = _chunks(s_pad, 512)
        windows = [(qc, pr) for qc in range(2) for pr in range(4)]
        NW = len(windows)
        nstep = NW * nsk

        units = []  # (unit, earliest_step, deadline_step)
        E_XKH1 = 3 if h_k < s_pad else 0  # xk 2nd half lands ~step 3
        E_XVP1 = 4 if p_v0 < s_pad else 0
        E_XQH1 = 8  # xq 2nd half lands ~step 8
        for t in range(nsk):
            e = 0 if t < 5 else E_XVP1
            units.append((("v", t), e, max(t, e)))  # used by av at step t+2
        for j in range(4):
            for ci, (c0, cw) in enumerate(kch):
                if j == 0 and ci == 0:
                    continue  # up-front
                use = 0 * nsk + c0 // 128 if j == 0 else j * nsk + c0 // 128
                e = E_XKH1 if c0 >= h_k else 0
                units.append((("k", j, c0, cw), e, max(use - 2, e)))
            if j > 0:
                units.append((("q", j, 0, 512), 0, j * nsk - 2))
            units.append((("q", j, 512, 512), E_XQH1, max((4 + j) * nsk - 2, E_XQH1)))

        pinned = {}  # step -> list of units
        for qt in range(4):  # O(0) inside window (1,0), after finish(0,3)
            pinned.setdefault(4 * nsk + min(5 + qt, nsk - 1), []).append(("o", 0, qt))
        for qt in (0, 1):  # O(1) qt0/qt1 partial pre-accumulation in (1,3)
            pinned.setdefault(7 * nsk + min(5 + qt, nsk - 1), []).append(("o1p", 1, qt))

        # usable slots: window 0 all steps; later windows steps >= 4 (the
        # borrowed av set is released by the previous window's normalization
        # chain only ~3 steps in)
        free = []
        for wi in range(NW):
            s0 = 0 if wi == 0 else 4
            for t in range(s0, nsk):
                st = wi * nsk + t
                if st not in pinned:
                    free.append(st)
        cap = {st: (2 if st >= 3 else 1) if st < nsk else 1 for st in free}
        load = {st: 0 for st in free}
        plan = {st: [] for st in range(nstep)}
        for st, us in pinned.items():
            plan[st] = list(us)
        for u, e, dl in sorted(units, key=lambda x: x[2]):
            cands = [s for s in free if e <= s <= dl and load[s] < cap[s]]
            if cands:
                s = max(cands)  # latest-fit: don't front-load early windows
            else:
                cands = [s for s in free if e <= s <= dl]
                s = max(cands) if cands else min(s for s in free if s >= e)
            plan[s].append(u)
            load[s] += 1

        o1_ps = {}

        def run_unit(u):
            if u[0] == "v":
                v_proj(u[1])
            elif u[0] in ("k", "q"):
                kq_chunk(u[0], u[1], u[2], u[3])
            elif u[0] == "o":
                o_unit(u[1], u[2])
            elif u[0] == "o1p":  # O(1) partial pre-accumulation (pr 0..2)
                o1_ps[u[2]] = o_unit(u[1], u[2], prs=range(3), stop=False)

        # ---- up-front projections ----
        kq_chunk("k", 0, 0, 512)
        kq_chunk("q", 0, 0, 512)

        # ---- continuous attention pipeline ----
        AVLAG = 2
        wctx = {}  # wi -> (avA, avB, hA, hB, q0)

        def emit_av(wi, t, wx):
            avA, avB, hA, hB, q0 = wctx[wi]
            last = t == nsk - 1
            nc.tensor.matmul(
                avA[0:65, :],
                lhsT=v[t][:, hA : hA + 1, 0 : DH + 1],
                rhs=wx[:, 0:512],
                start=(t == 0),
                stop=last,
            )
            nc.tensor.matmul(
                avB[0:65, :],
                lhsT=v[t][:, hB : hB + 1, 0 : DH + 1],
                rhs=wx[:, 512:1024],
                start=(t == 0),
                stop=last,
            )
            if last:
                finish(wi)

        def finish(wi):
            avA, avB, hA, hB, q0 = wctx.pop(wi)
            qc, pr = windows[wi]
            # normalization (reciprocal needs SBUF input: copy denom rows out)
            dsA = r_p.tile([1, 512], f32, tag="dsA", name=f"dsA{qc}{pr}")
            dsB = r_p.tile([1, 512], f32, tag="dsB", name=f"dsB{qc}{pr}")
            nc.vector.tensor_copy(dsA[0:1, :], avA[64:65, :])
            nc.vector.tensor_copy(dsB[0:1, :], avB[64:65, :])
            rfA = r_p.tile([1, 512], f32, tag="rfA", name=f"rfA{qc}{pr}")
            rfB = r_p.tile([1, 512], f32, tag="rfB", name=f"rfB{qc}{pr}")
            nc.vector.reciprocal_approx_fast(rfA[0:1, :], dsA[0:1, :])
            nc.vector.reciprocal_approx_fast(rfB[0:1, :], dsB[0:1, :])
            bcsA = r_p.tile([64, 512], f32, tag="bcsA", name=f"bcsA{qc}{pr}")
            bcsB = r_p.tile([64, 512], f32, tag="bcsB", name=f"bcsB{qc}{pr}")
            nc.gpsimd.partition_broadcast(bcsA[0:64, :], rfA[0:1, :], channels=64)
            nc.gpsimd.partition_broadcast(bcsB[0:64, :], rfB[0:1, :], channels=64)
            nc.vector.tensor_mul(
                attnN[pr][0:64, q0 : q0 + 512], avA[0:64, :], bcsA[0:64, :]
            )
            nc.vector.tensor_mul(
                attnN[pr][64:128, q0 : q0 + 512], avB[0:64, :], bcsB[0:64, :]
            )

        hist = []  # emitted (wi, t, wx) awaiting their av pair
        for step in range(nstep):
            wi, t = divmod(step, nsk)
            qc, pr = windows[wi]
            if t == 0:
                st = wi % 2
                wctx[wi] = (
                    ps_av.tile([65, 512], f32, tag=f"av{st}A", name=f"avA{qc}{pr}"),
                    ps_av.tile([65, 512], f32, tag=f"av{st}B", name=f"avB{qc}{pr}"),
                    2 * pr,
                    2 * pr + 1,
                    qc * 512,
                )
            bctx["set"] = 1 - (wi % 2)
            lg = ps_lg.tile([128, 1024], f32, tag="lg", name=f"lg{qc}{pr}_{t}")
            q0 = qc * 512
            nc.tensor.matmul(
                lg[:, 0:512],
                lhsT=kT[pr][0:64, t * 128 : (t + 1) * 128],
                rhs=qT[pr][0:64, q0 : q0 + 512],
                start=True,
                stop=True,
            )
            nc.tensor.matmul(
                lg[:, 512:1024],
                lhsT=kT[pr][64:128, t * 128 : (t + 1) * 128],
                rhs=qT[pr][64:128, q0 : q0 + 512],
                start=True,
                stop=True,
            )
            wx = wexp_p.tile([128, 1024], mmdt, tag="wexp", name=f"wx{qc}{pr}_{t}")
            nc.scalar.activation(wx[:], lg[:], EXP, bias=mb[:, t : t + 1], scale=SCALE)
            hist.append((wi, t, wx))
            lag = 1 if step >= nstep - 2 else AVLAG
            while len(hist) > lag:
                emit_av(*hist.pop(0))
            for u in plan[step]:
                run_unit(u)
        # qt2 partial on the first now-free lg slot (runs while the last
        # exps drain), then flush the remaining av pairs + final finish
        o1_ps[2] = o_unit(
            1, 2,
            ps=ps_lg.tile([128, 1024], f32, tag="lg", name="o1f2")[:, 0:512],
            prs=range(3), stop=False,
        )
        for h in hist:
            emit_av(*h)
        o1_ps[3] = o_unit(
            1, 3,
            ps=ps_lg.tile([128, 1024], f32, tag="lg", name="o1f3")[:, 0:512],
            prs=range(3), stop=False,
        )

        # ---- tail: finalize O(1): one matmul (pr=3) + drain per qt ----
        for qt in range(4):
            ps = o1_ps[qt]
            qq = 512 + qt * 128
            nc.tensor.matmul(
                ps[:],
                lhsT=attnN[3][:, qq : qq + 128],
                rhs=blo[:, 3 * 512 : 4 * 512],
                start=False,
                stop=True,
            )
            o_drain(1, qt, ps)

    nc.compile()
    return nc


def _prep_inputs(query, key, value, mask, wq_w, wq_b, wk_w, wk_b, wv_w, wv_b, wo_w, wo_b,
                 mm_dtype="bf16"):
    import ml_dtypes

    od = {"bf16": ml_dtypes.bfloat16, "f32r": np.float32, "f32": np.float32}[mm_dtype]
    f = lambda a: np.ascontiguousarray(np.asarray(a, dtype=np.float32))
    g = lambda a: np.ascontiguousarray(np.asarray(a).astype(od))
    query, key, value = f(query), f(key), f(value)
    wq_w, wk_w, wv_w, wo_w = f(wq_w), f(wk_w), f(wv_w), f(wo_w)
    mask = np.asarray(mask)

    keeps = [np.flatnonzero(mask[b] == 0) for b in range(B)]
    cnts = [len(k) for k in keeps]
    assert min(cnts) > 0, "all-masked batch not supported"
    s_pad = max(128, ((max(cnts) + 127) // 128) * 128)
    nsk = s_pad // 128

    bq_pp = np.ascontiguousarray(f(wq_b).reshape(4, 128).T)
    bk_pp = np.ascontiguousarray(f(wk_b).reshape(4, 128).T)
    bvb = np.broadcast_to(f(wv_b).reshape(1, D), (128, D))
    bob = np.broadcast_to(f(wo_b).reshape(1, D), (128, D))

    def wchunks(w):
        # [512, 512] -> [128, 4*512]: col block dk holds rows dk*128..dk*128+128
        return w.reshape(4, 128, D).transpose(1, 0, 2).reshape(128, 4 * D)

    def xchunks(xt):
        # [512, S] -> [128, 4*S]
        s = xt.shape[1]
        return xt.reshape(4, 128, s).transpose(1, 0, 2).reshape(128, 4 * s)

    blob_o = np.concatenate([wchunks(f(wo_w)), bob], axis=1)

    common = dict(
        bq_pp=bq_pp, bk_pp=bk_pp,
        blob_o=g(blob_o),
    )
    in_maps = []
    for b in range(B):
        kc = np.zeros((s_pad, D), np.float32)
        kc[: cnts[b]] = key[b][keeps[b]]
        vc = np.zeros((s_pad, D), np.float32)
        vc[: cnts[b]] = value[b][keeps[b]]
        blob_k = g(np.concatenate([wchunks(f(wk_w)), xchunks(kc.T)], axis=1))
        blob_v = g(
            np.concatenate(
                [
                    wchunks(f(wv_w)),
                    xchunks(vc.T),
                    bvb,
                    np.ones((128, 8), np.float32),
                ],
                axis=1,
            )
        )
        mbf = np.zeros(s_pad, np.float32)
        mbf[cnts[b] :] = -1e9
        mbd = np.ascontiguousarray(mbf.reshape(nsk, 128).T)
        for qh in range(2):
            blob_q = g(
                np.concatenate(
                    [wchunks(f(wq_w)), xchunks(query[b, qh * SQ : (qh + 1) * SQ, :].T)],
                    axis=1,
                )
            )
            in_maps.append(
                dict(blob_k=blob_k, blob_q=blob_q, blob_v=blob_v, mb=mbd, **common)
            )
    return s_pad, in_maps


def kernel(**inputs):
    from concourse import bass_utils

    mmd = os.environ.get("BASSK_MMDT", "bf16")
    s_pad, in_maps = _prep_inputs(**inputs, mm_dtype=mmd)
    key = (s_pad, mmd)
    if key not in _BUILT:
        _BUILT[key] = build_bass(s_pad, mm_dtype=key[1])
    nc = _BUILT[key]
    kw = {}
    if os.environ.get("BASSK_TRACE"):
        kw = dict(trace=True, stitch_traces=False)
    res = bass_utils.run_bass_kernel_spmd(nc, in_maps, core_ids=list(range(NCORE)), **kw)
    out = np.empty((B, S, D), np.float32)
    for c in range(NCORE):
        b, qh = c // 2, c % 2
        out[b, qh * SQ : (qh + 1) * SQ, :] = res.results[c]["out"]
    kernel.last_result = res
    return out
